# revision 1
# baseline (speedup 1.0000x reference)
"""DMN forward on 8 Trainium2 NeuronCores (Bass/Tile).

Sharding: batch rows 8/core for fact+question encoding and episodic memory
(core j owns batch rows 8j..8j+7 and their 160 fact sequences); decode GRU
replicated on all cores, fc/log-softmax vocab-sharded 4000 columns/core, with
one tiny AllGather per decode step for the greedy-argmax feedback and one at
the end for the softmax normalizers.

All matmuls on the recurrent chain run in fp32 on the PE (measured ~4e-7
faithful to numpy fp32); sigmoid is computed in tanh form to match XLA's
logistic lowering. The fc matmul is fp32 too so the argmax needs no rescoring.
"""

import os
import numpy as np

import concourse.bass as bass
import concourse.bacc as bacc
import concourse.mybir as mybir
from concourse.tile import TileContext
from concourse.bass_utils import run_bass_kernel_spmd
from concourse.masks import make_identity

AF = mybir.ActivationFunctionType
ALU = mybir.AluOpType
DT = mybir.dt

V, E, H = 32000, 256, 256
B, NF, FL, QL = 64, 20, 32, 16
N_EPISODE = 3
SEQBEGIN = 1
NCORE = 8
BB = B // NCORE            # batch rows per core = 8
NSEQ = BB * NF             # fact seqs per core = 160
NTOK = NSEQ * FL           # fact tokens per core = 5120
VS = V // NCORE            # vocab shard = 4000
NCHUNK = 8
CHW = VS // NCHUNK         # 500

GK = {"ig": E, "qg": E, "att": H, "mem": H, "ans": 2 * H}


def build_nc(alen, fcb_nonzero):
    nc = bacc.Bacc("TRN2", num_devices=NCORE)

    def dram_in(name, shape, dtype=DT.float32):
        return nc.dram_tensor(name, list(shape), dtype, kind="ExternalInput")

    io = {}
    io["facts_idx"] = dram_in("facts_idx", [NTOK, 1], DT.int32)
    io["q_idx"] = dram_in("q_idx", [BB * QL, 1], DT.int32)
    io["embed"] = dram_in("embed", [V, E])
    io["fcwT"] = dram_in("fcwT", [E, VS])
    io["last0T"] = dram_in("last0T", [E, B])
    io["voff"] = dram_in("voff", [B, 1])
    if fcb_nonzero:
        io["fcb"] = dram_in("fcb", [B, VS])
    for g, kin in GK.items():
        io[f"{g}_wihT"] = dram_in(f"{g}_wihT", [kin, 3 * H])
        io[f"{g}_whhT"] = dram_in(f"{g}_whhT", [H, 3 * H])
        io[f"{g}_hbrz"] = dram_in(f"{g}_hbrz", [128, 4])
        io[f"{g}_bin"] = dram_in(f"{g}_bin", [128, 2])
        io[f"{g}_bhn"] = dram_in(f"{g}_bhn", [128, 2])
    io["g1T"] = dram_in("g1T", [4 * H, H])
    io["g2T"] = dram_in("g2T", [H, 1])
    io["gb1"] = dram_in("gb1", [128, 2])
    io["gb2h"] = dram_in("gb2h", [1, 1])

    out_logp = nc.dram_tensor("out_logp", [B * alen, VS], DT.float32, kind="ExternalOutput")

    cc_enc_in = nc.dram_tensor("cc_enc_in", [BB, 2 * H], DT.float32, kind="Internal")
    cc_enc_out = nc.dram_tensor("cc_enc_out", [B, 2 * H], DT.float32, kind="Internal", addr_space="Shared")
    cc_top_in = [nc.dram_tensor(f"cc_top_in{t}", [B, 2], DT.float32, kind="Internal") for t in range(alen)]
    cc_top_out = [nc.dram_tensor(f"cc_top_out{t}", [NCORE * B, 2], DT.float32, kind="Internal",
                                 addr_space="Shared") for t in range(alen)]
    cc_s_in = nc.dram_tensor("cc_s_in", [B, alen], DT.float32, kind="Internal")
    lst_dram = [nc.dram_tensor(f"lst_dram{t}", [B, VS], DT.bfloat16, kind="Internal") for t in range(alen)]
    cc_s_out = nc.dram_tensor("cc_s_out", [NCORE * B, alen], DT.float32, kind="Internal", addr_space="Shared")
    gid = [nc.dram_tensor(f"gid{m}", [128, NTOK], DT.float32, kind="Internal") for m in range(6)]
    rg = [list(range(NCORE))]

    with TileContext(nc) as tc:
        with tc.tile_pool(name="shared", bufs=1) as shp, \
             tc.tile_pool(name="state", bufs=1) as st, \
             tc.tile_pool(name="work", bufs=3) as wk, \
             tc.tile_pool(name="ps", bufs=8, space="PSUM") as ps:

            ident = shp.tile([128, 128], DT.float32)
            make_identity(nc, ident[:, :])
            zb = shp.tile([128, 1], DT.float32)
            nc.vector.memset(zb[:, :], 0.0)

            W = {}

            def load_w(pool, g):
                kin = GK[g]
                W[f"{g}_wihT"] = []
                for k in range(kin // 128):
                    t = pool.tile([128, 3 * H], DT.float32, name=f"{g}wih{k}")
                    nc.sync.dma_start(t[:, :], io[f"{g}_wihT"][k * 128:(k + 1) * 128, :])
                    W[f"{g}_wihT"].append(t)
                W[f"{g}_whhT"] = []
                for k in range(2):
                    t = pool.tile([128, 3 * H], DT.float32, name=f"{g}whh{k}")
                    nc.sync.dma_start(t[:, :], io[f"{g}_whhT"][k * 128:(k + 1) * 128, :])
                    W[f"{g}_whhT"].append(t)
                for bn, w in (("hbrz", 4), ("bin", 2), ("bhn", 2)):
                    t = pool.tile([128, w], DT.float32, name=f"{g}{bn}")
                    nc.sync.dma_start(t[:, :], io[f"{g}_{bn}"][:, :])
                    W[f"{g}_{bn}"] = t

            evict_rr = [0]

            def evict(dst_ap, src_ap):
                if evict_rr[0] % 2 == 0:
                    nc.vector.tensor_copy(dst_ap, src_ap)
                else:
                    nc.scalar.activation(dst_ap, src_ap, AF.Copy)
                evict_rr[0] += 1

            # ---------------- GRU step (transposed layout) ----------------
            def gru_step(g, hT, rhs_x, n_free, name="", gi_sb=None, xw_nk=None):
                """gi_sb: optional 6 SBUF APs [128, n_free] of precomputed x-proj
                (T-layout gate tiles); then x-MMs cover only rhs_x (may be [])."""
                xw = W[f"{g}_wihT"]
                hw = W[f"{g}_whhT"]
                have_x = len(rhs_x) > 0
                prz = [ps.tile([128, n_free], DT.float32, tag="bank", name=f"{name}prz{m}") for m in range(4)]
                pni = [ps.tile([128, n_free], DT.float32, tag="bank", name=f"{name}pni{m}")
                       for m in range(2)] if have_x else None
                pnh = [ps.tile([128, n_free], DT.float32, tag="bank", name=f"{name}pnh{m}") for m in range(2)]

                def mm(dst, lhsT_tiles, rhs_list, m, first, last):
                    nk = len(rhs_list)
                    for k in range(nk):
                        nc.tensor.matmul(dst[:, :], lhsT_tiles[k][:, m * 128:(m + 1) * 128],
                                         rhs_list[k], start=(first and k == 0),
                                         stop=(last and k == nk - 1))

                for m in range(4):
                    if have_x:
                        mm(prz[m], xw, rhs_x, m, True, False)
                        mm(prz[m], hw, [t[:, :] for t in hT], m, False, True)
                    else:
                        mm(prz[m], hw, [t[:, :] for t in hT], m, True, True)
                for m in range(2):
                    if have_x:
                        mm(pni[m], xw, rhs_x, 4 + m, True, True)
                    mm(pnh[m], hw, [t[:, :] for t in hT], 4 + m, True, True)

                hbrz, bin_, bhn = W[f"{g}_hbrz"], W[f"{g}_bin"], W[f"{g}_bhn"]
                hnew = []
                for hf in range(2):
                    if gi_sb is not None:
                        prin = wk.tile([128, n_free], DT.float32, tag=f"gpr{n_free}", bufs=2,
                                       name=f"{name}pr{hf}")
                        nc.vector.tensor_add(prin[:, :], prz[hf][:, :], gi_sb[hf])
                        przr = prin[:, :]
                        pzin = wk.tile([128, n_free], DT.float32, tag=f"gpz{n_free}", bufs=2,
                                       name=f"{name}pz{hf}")
                        nc.vector.tensor_add(pzin[:, :], prz[2 + hf][:, :], gi_sb[2 + hf])
                        przz = pzin[:, :]
                    else:
                        przr = prz[hf][:, :]
                        przz = prz[2 + hf][:, :]
                    tr = wk.tile([128, n_free], DT.float32, tag=f"gtr{n_free}", name=f"{name}tr{hf}")
                    nc.scalar.activation(tr[:, :], przr, AF.Tanh,
                                         bias=hbrz[:, hf:hf + 1], scale=0.5)
                    r = wk.tile([128, n_free], DT.float32, tag=f"gr{n_free}", name=f"{name}r{hf}")
                    nc.vector.tensor_scalar(r[:, :], tr[:, :], 0.5, 0.5, ALU.mult, ALU.add)
                    tz = wk.tile([128, n_free], DT.float32, tag=f"gtz{n_free}", name=f"{name}tz{hf}")
                    nc.scalar.activation(tz[:, :], przz, AF.Tanh,
                                         bias=hbrz[:, 2 + hf:3 + hf], scale=0.5)
                    z = wk.tile([128, n_free], DT.float32, tag=f"gz{n_free}", name=f"{name}z{hf}")
                    nc.vector.tensor_scalar(z[:, :], tz[:, :], 0.5, 0.5, ALU.mult, ALU.add)
                    y = wk.tile([128, n_free], DT.float32, tag=f"gy{n_free}", name=f"{name}y{hf}")
                    nc.vector.scalar_tensor_tensor(y[:, :], pnh[hf][:, :], bhn[:, hf:hf + 1],
                                                   r[:, :], ALU.add, ALU.mult)
                    u = wk.tile([128, n_free], DT.float32, tag=f"gu{n_free}", name=f"{name}u{hf}")
                    if gi_sb is not None and pni is None:
                        nc.vector.scalar_tensor_tensor(u[:, :], gi_sb[4 + hf], bin_[:, hf:hf + 1],
                                                       y[:, :], ALU.add, ALU.add)
                    elif gi_sb is not None:
                        u1 = wk.tile([128, n_free], DT.float32, tag=f"gu1{n_free}", bufs=2,
                                     name=f"{name}u1{hf}")
                        nc.vector.scalar_tensor_tensor(u1[:, :], pni[hf][:, :], bin_[:, hf:hf + 1],
                                                       y[:, :], ALU.add, ALU.add)
                        nc.vector.tensor_add(u[:, :], u1[:, :], gi_sb[4 + hf])
                    else:
                        nc.vector.scalar_tensor_tensor(u[:, :], pni[hf][:, :], bin_[:, hf:hf + 1],
                                                       y[:, :], ALU.add, ALU.add)
                    n = wk.tile([128, n_free], DT.float32, tag=f"gn{n_free}", name=f"{name}n{hf}")
                    nc.scalar.activation(n[:, :], u[:, :], AF.Tanh, bias=zb[:, :], scale=1.0)
                    d = wk.tile([128, n_free], DT.float32, tag=f"gd{n_free}", name=f"{name}d{hf}")
                    nc.vector.tensor_sub(d[:, :], hT[hf][:, :], n[:, :])
                    w2 = wk.tile([128, n_free], DT.float32, tag=f"gw{n_free}", name=f"{name}w{hf}")
                    nc.vector.tensor_mul(w2[:, :], z[:, :], d[:, :])
                    hn = wk.tile([128, n_free], DT.float32, tag=f"ghn{n_free}", bufs=4, name=f"{name}hn{hf}")
                    nc.vector.tensor_add(hn[:, :], n[:, :], w2[:, :])
                    hnew.append(hn)
                return hnew

            dbg = int(os.environ.get("K_DEBUG_STEPS", "0"))
            n_fl = dbg or FL
            n_ql = dbg or QL
            n_nf = dbg or NF
            n_ep = 1 if dbg else N_EPISODE

            # ================= P1+P2: facts =================
            with tc.tile_pool(name="fpool", bufs=1) as fp:
                load_w(fp, "ig")
                load_w(fp, "qg")
                XT = [fp.tile([128, NTOK], DT.float32, name=f"XT{k}") for k in range(2)]
                fidx = fp.tile([128, NTOK // 128], DT.int32, name="fidx")
                nc.sync.dma_start(fidx[:, :], io["facts_idx"].rearrange("(b a) o -> a (b o)", a=128))
                for i in range(NTOK // 128):
                    gt = wk.tile([128, E], DT.float32, tag="fgat", bufs=4, name=f"fg{i}")
                    nc.gpsimd.indirect_dma_start(
                        out=gt[:, :], out_offset=None, in_=io["embed"][:, :],
                        in_offset=bass.IndirectOffsetOnAxis(ap=fidx[:, i:i + 1], axis=0),
                    )
                    for ch in range(2):
                        pt = ps.tile([128, 128], DT.float32, tag="bank", name=f"ftp{i}_{ch}")
                        nc.tensor.transpose(pt[:, :], gt[:, ch * 128:(ch + 1) * 128], ident[:, :])
                        evict(XT[ch][:, i * 128:(i + 1) * 128], pt[:, :])

                # hoisted x-projection: gi = X @ wih.T, streamed through DRAM.
                # XT/gid columns are time-major: token index = t*NSEQ + s.
                NCH = NTOK // 512
                for m in range(6):
                    for c in range(NCH):
                        pp = ps.tile([128, 512], DT.float32, tag="bank", name=f"xp{m}_{c}")
                        for k in range(2):
                            nc.tensor.matmul(pp[:, :], W["ig_wihT"][k][:, m * 128:(m + 1) * 128],
                                             XT[k][:, c * 512:(c + 1) * 512],
                                             start=(k == 0), stop=(k == 1))
                        stg = wk.tile([128, 512], DT.float32, tag="gst", bufs=4, name=f"gst{m}_{c}")
                        evict(stg[:, :], pp[:, :])
                        nc.sync.dma_start(gid[m][:, c * 512:(c + 1) * 512], stg[:, :])

                hT = [st.tile([128, NSEQ], DT.float32, name=f"hT{k}") for k in range(2)]
                for t in hT:
                    nc.vector.memset(t[:, :], 0.0)
                hT = [t[:, :] for t in hT]
                for step in range(n_fl):
                    git = []
                    for m in range(6):
                        gt_ = wk.tile([128, NSEQ], DT.float32, tag=f"git{m}", bufs=2,
                                      name=f"git{step}_{m}")
                        nc.sync.dma_start(gt_[:, :], gid[m][:, step * NSEQ:(step + 1) * NSEQ])
                        git.append(gt_[:, :])
                    hnew = gru_step("ig", hT, [], NSEQ, name=f"f{step}_", gi_sb=git)
                    hT = [t[:, :] for t in hnew]
                # persist enc_facts
                encfT = [st.tile([128, NSEQ], DT.float32, name=f"encfT{k}") for k in range(2)]
                for k in range(2):
                    nc.vector.tensor_copy(encfT[k][:, :], hT[k])

                # ================= P3: questions =================
                qidx = wk.tile([128, 1], DT.int32, name="qidx")
                nc.sync.dma_start(qidx[:, :], io["q_idx"][:, :])
                qg_t = wk.tile([128, E], DT.float32, tag="fgat", bufs=4, name="qgat")
                nc.gpsimd.indirect_dma_start(
                    out=qg_t[:, :], out_offset=None, in_=io["embed"][:, :],
                    in_offset=bass.IndirectOffsetOnAxis(ap=qidx[:, :1], axis=0),
                )
                XQT = [st.tile([128, BB * QL], DT.float32, name=f"XQT{k}") for k in range(2)]
                for ch in range(2):
                    pt = ps.tile([128, 128], DT.float32, tag="bank", name=f"qtp{ch}")
                    nc.tensor.transpose(pt[:, :], qg_t[:, ch * 128:(ch + 1) * 128], ident[:, :])
                    evict(XQT[ch][:, :], pt[:, :])
                giq = []
                for m in range(6):
                    pp = ps.tile([128, BB * QL], DT.float32, tag="bank", name=f"qxp{m}")
                    for k in range(2):
                        nc.tensor.matmul(pp[:, :], W["qg_wihT"][k][:, m * 128:(m + 1) * 128],
                                         XQT[k][:, :], start=(k == 0), stop=(k == 1))
                    gq = st.tile([128, BB * QL], DT.float32, name=f"giq{m}")
                    evict(gq[:, :], pp[:, :])
                    giq.append(gq)
                hq = [st.tile([128, BB], DT.float32, name=f"hqT{k}") for k in range(2)]
                for t in hq:
                    nc.vector.memset(t[:, :], 0.0)
                hq = [t[:, :] for t in hq]
                for step in range(n_ql):
                    gis = [giq[m][:, step * BB:(step + 1) * BB] for m in range(6)]
                    hnew = gru_step("qg", hq, [], BB, name=f"q{step}_", gi_sb=gis)
                    hq = [t[:, :] for t in hnew]
                hqT = [st.tile([128, BB], DT.float32, name=f"hqTf{k}") for k in range(2)]
                for k in range(2):
                    nc.vector.tensor_copy(hqT[k][:, :], hq[k])

            # ================= P4: episodes =================
            with tc.tile_pool(name="epool", bufs=1) as epl:
                load_w(epl, "att")
                load_w(epl, "mem")
                g1T = []
                for k in range(8):
                    t = epl.tile([128, H], DT.float32, name=f"g1T{k}")
                    nc.sync.dma_start(t[:, :], io["g1T"][k * 128:(k + 1) * 128, :])
                    g1T.append(t)
                g2T = []
                for k in range(2):
                    t = epl.tile([128, 1], DT.float32, name=f"g2T{k}")
                    nc.sync.dma_start(t[:, :], io["g2T"][k * 128:(k + 1) * 128, :])
                    g2T.append(t)
                gb1 = epl.tile([128, 2], DT.float32)
                nc.sync.dma_start(gb1[:, :], io["gb1"][:, :])
                gb2h = epl.tile([1, 1], DT.float32)
                nc.sync.dma_start(gb2h[:, :], io["gb2h"][:, :])

                memT = [st.tile([128, BB], DT.float32, name=f"memT{k}") for k in range(2)]
                for k in range(2):
                    nc.vector.tensor_copy(memT[k][:, :], hqT[k][:, :])
                memT_ap = [t[:, :] for t in memT]

                encf3 = [encfT[k][:, :].rearrange("p (r i) -> p r i", i=NF) for k in range(2)]
                gia = []
                for m in range(6):
                    pp = ps.tile([128, NSEQ], DT.float32, tag="bank", name=f"axp{m}")
                    for k in range(2):
                        nc.tensor.matmul(pp[:, :], W["att_wihT"][k][:, m * 128:(m + 1) * 128],
                                         encfT[k][:, :], start=(k == 0), stop=(k == 1))
                    ga = epl.tile([128, NSEQ], DT.float32, name=f"gia{m}")
                    evict(ga[:, :], pp[:, :])
                    gia.append(ga)
                gia3 = [g[:, :].rearrange("p (r i) -> p r i", i=NF) for g in gia]
                for ep in range(n_ep):
                    ZT = [wk.tile([128, NSEQ], DT.float32, tag="zt", bufs=8, name=f"ZT{ep}_{x}")
                          for x in range(8)]
                    for ch in range(2):
                        qb = hqT[ch][:, :].to_broadcast([128, BB, NF])
                        mb = memT_ap[ch].to_broadcast([128, BB, NF])
                        z3 = [ZT[x][:, :].rearrange("p (r i) -> p r i", i=NF) for x in range(8)]
                        nc.vector.tensor_mul(z3[0 + ch], encf3[ch], qb)
                        nc.vector.tensor_mul(z3[2 + ch], encf3[ch], mb)
                        dq = wk.tile([128, NSEQ], DT.float32, tag="dq", name=f"dq{ep}_{ch}")
                        nc.vector.tensor_sub(dq[:, :].rearrange("p (r i) -> p r i", i=NF), encf3[ch], qb)
                        nc.scalar.activation(ZT[4 + ch][:, :], dq[:, :], AF.Abs)
                        dm = wk.tile([128, NSEQ], DT.float32, tag="dm", name=f"dm{ep}_{ch}")
                        nc.vector.tensor_sub(dm[:, :].rearrange("p (r i) -> p r i", i=NF), encf3[ch], mb)
                        nc.scalar.activation(ZT[6 + ch][:, :], dm[:, :], AF.Abs)
                    p1T = []
                    for m in range(2):
                        pp = ps.tile([128, NSEQ], DT.float32, tag="bank", name=f"p1{ep}_{m}")
                        for k in range(8):
                            nc.tensor.matmul(pp[:, :], g1T[k][:, m * 128:(m + 1) * 128], ZT[k][:, :],
                                             start=(k == 0), stop=(k == 7))
                        t1 = wk.tile([128, NSEQ], DT.float32, tag="p1s", bufs=2, name=f"p1s{ep}_{m}")
                        nc.scalar.activation(t1[:, :], pp[:, :], AF.Tanh, bias=gb1[:, m:m + 1], scale=1.0)
                        p1T.append(t1)
                    pgp = ps.tile([1, NSEQ], DT.float32, tag="bank", name=f"pg{ep}")
                    for k in range(2):
                        nc.tensor.matmul(pgp[:, :], g2T[k][:, :], p1T[k][:, :], start=(k == 0), stop=(k == 1))
                    tg = wk.tile([1, NSEQ], DT.float32, tag="tg", name=f"tg{ep}")
                    nc.scalar.activation(tg[:, :], pgp[:, :], AF.Tanh, bias=gb2h[:, :], scale=0.5)
                    g_row = wk.tile([1, NSEQ], DT.float32, tag="grow", name=f"grow{ep}")
                    nc.vector.tensor_scalar(g_row[:, :], tg[:, :], 0.5, 0.5, ALU.mult, ALU.add)
                    gB = wk.tile([128, NSEQ], DT.float32, tag="gB", name=f"gB{ep}")
                    nc.gpsimd.partition_broadcast(gB[:, :], g_row[:, :])
                    gB3 = gB[:, :].rearrange("p (r i) -> p r i", i=NF)

                    eT = [wk.tile([128, BB], DT.float32, tag=f"eT{k}", bufs=2, name=f"eT{ep}_{k}")
                          for k in range(2)]
                    for t in eT:
                        nc.vector.memset(t[:, :], 0.0)
                    eT = [t[:, :] for t in eT]
                    for i in range(n_nf):
                        gis = [gia3[m][:, :, i] for m in range(6)]
                        enew = gru_step("att", eT, [], BB, name=f"e{ep}_{i}_", gi_sb=gis)
                        e2 = []
                        for k in range(2):
                            dd = wk.tile([128, BB], DT.float32, tag="edd", name=f"edd{ep}_{i}_{k}")
                            nc.vector.tensor_sub(dd[:, :], enew[k][:, :], eT[k])
                            gp = wk.tile([128, BB], DT.float32, tag="egp", name=f"egp{ep}_{i}_{k}")
                            nc.vector.tensor_mul(gp[:, :], gB3[:, :, i], dd[:, :])
                            en = wk.tile([128, BB], DT.float32, tag="enx", bufs=4, name=f"enx{ep}_{i}_{k}")
                            nc.vector.tensor_add(en[:, :], eT[k], gp[:, :])
                            e2.append(en[:, :])
                        eT = e2
                    mnew = gru_step("mem", memT_ap, eT, BB, name=f"m{ep}_")
                    memT_ap = [t[:, :] for t in mnew]

                # persist memory into state pool
                memF = [st.tile([128, BB], DT.float32, name=f"memF{k}") for k in range(2)]
                for k in range(2):
                    nc.vector.tensor_copy(memF[k][:, :], memT_ap[k])

            # ================= P5: all-gather mem|enc_q =================
            encrow = wk.tile([BB, 2 * H], DT.float32, name="encrow")
            for ch in range(2):
                pt = ps.tile([BB, 128], DT.float32, tag="bank", name=f"egm{ch}")
                nc.tensor.transpose(pt[:, :], memF[ch][:, :], ident[:, :])
                evict(encrow[:, ch * 128:(ch + 1) * 128], pt[:, :])
                pt2 = ps.tile([BB, 128], DT.float32, tag="bank", name=f"egq{ch}")
                nc.tensor.transpose(pt2[:, :], hqT[ch][:, :], ident[:, :])
                evict(encrow[:, 256 + ch * 128:256 + (ch + 1) * 128], pt2[:, :])
            nc.sync.dma_start(cc_enc_in[:, :], encrow[:, :])
            nc.gpsimd.collective_compute("AllGather", ALU.bypass, ins=[cc_enc_in[:, :]],
                                         outs=[cc_enc_out[:, :]], replica_groups=rg)
            enc_all = wk.tile([B, 2 * H], DT.float32, name="enc_all")
            nc.sync.dma_start(enc_all[:, :], cc_enc_out[:, :])

            with tc.tile_pool(name="dpool", bufs=1) as dp:
                load_w(dp, "ans")
                fcwT = []
                for k in range(2):
                    t = dp.tile([128, VS], DT.float32, name=f"fcwT{k}")
                    nc.sync.dma_start(t[:, :], io["fcwT"][k * 128:(k + 1) * 128, :])
                    fcwT.append(t)
                last0T = []
                for k in range(2):
                    t = dp.tile([128, B], DT.float32, name=f"l0T{k}")
                    nc.sync.dma_start(t[:, :], io["last0T"][k * 128:(k + 1) * 128, :])
                    last0T.append(t)
                vofft = dp.tile([B, 1], DT.float32)
                nc.sync.dma_start(vofft[:, :], io["voff"][:, :])
                iota_i = dp.tile([B, CHW], DT.int32)
                nc.gpsimd.iota(iota_i[:, :], pattern=[[1, CHW]], base=0, channel_multiplier=0)
                iota_f = dp.tile([B, CHW], DT.float32)
                nc.vector.tensor_copy(iota_f[:, :], iota_i[:, :])
                ch_i = dp.tile([B, NCHUNK], DT.int32)
                nc.gpsimd.iota(ch_i[:, :], pattern=[[1, NCHUNK]], base=0, channel_multiplier=0)
                ch_f = dp.tile([B, NCHUNK], DT.float32)
                nc.vector.tensor_copy(ch_f[:, :], ch_i[:, :])
                stab = dp.tile([B, alen], DT.float32)

                # transposed views of gathered mem/enc_q
                memA = []
                qA = []
                for ch in range(2):
                    pt = ps.tile([128, B], DT.float32, tag="bank", name=f"tmA{ch}")
                    nc.tensor.transpose(pt[:, :], enc_all[:, ch * 128:(ch + 1) * 128], ident[:B, :B])
                    t = dp.tile([128, B], DT.float32, name=f"memA{ch}")
                    evict(t[:, :], pt[:, :])
                    memA.append(t)
                    pt2 = ps.tile([128, B], DT.float32, tag="bank", name=f"tqA{ch}")
                    nc.tensor.transpose(pt2[:, :], enc_all[:, 256 + ch * 128:256 + (ch + 1) * 128],
                                        ident[:B, :B])
                    t2 = dp.tile([128, B], DT.float32, name=f"qA{ch}")
                    evict(t2[:, :], pt2[:, :])
                    qA.append(t2)

                # ================= P6: decode =================
                # enc_q part of the ans-GRU input projection, once
                giqA = []
                for m in range(6):
                    pp = ps.tile([128, B], DT.float32, tag="bank", name=f"dxp{m}")
                    for k in range(2):
                        nc.tensor.matmul(pp[:, :], W["ans_wihT"][2 + k][:, m * 128:(m + 1) * 128],
                                         qA[k][:, :], start=(k == 0), stop=(k == 1))
                    gqa = dp.tile([128, B], DT.float32, name=f"giqA{m}")
                    evict(gqa[:, :], pp[:, :])
                    giqA.append(gqa)
                ans_xw_last = W["ans_wihT"][:2]
                hidT = [t[:, :] for t in memA]
                lastT = [t[:, :] for t in last0T]
                n_dec = alen if not dbg else min(alen, dbg)
                for t_step in range(n_dec):
                    W["ansl_wihT"] = ans_xw_last
                    W["ansl_whhT"] = W["ans_whhT"]
                    W["ansl_hbrz"] = W["ans_hbrz"]
                    W["ansl_bin"] = W["ans_bin"]
                    W["ansl_bhn"] = W["ans_bhn"]
                    hnew = gru_step("ansl", hidT, [lastT[0], lastT[1]], B,
                                    name=f"a{t_step}_", gi_sb=[g[:, :] for g in giqA])
                    hidT = [t[:, :] for t in hnew]

                    Mt = wk.tile([B, NCHUNK], DT.float32, tag="Mt", name=f"Mt{t_step}")
                    It = wk.tile([B, NCHUNK], DT.float32, tag="It", name=f"It{t_step}")
                    sext = wk.tile([B, NCHUNK], DT.float32, tag="sext", name=f"sext{t_step}")
                    for c in range(NCHUNK):
                        pl = ps.tile([B, CHW], DT.float32, tag="bank", name=f"pl{t_step}_{c}")
                        for k in range(2):
                            nc.tensor.matmul(pl[:, :], hidT[k], fcwT[k][:, c * CHW:(c + 1) * CHW],
                                             start=(k == 0), stop=(k == 1))
                        if fcb_nonzero:
                            # (unexpected path) fold bias via DVE add from a bcast tile
                            fcbt = wk.tile([B, CHW], DT.float32, tag="fcbt", name=f"fcbt{t_step}_{c}")
                            nc.sync.dma_start(fcbt[:, :], io["fcb"][:, c * CHW:(c + 1) * CHW])
                            nc.vector.tensor_add(pl[:, :], pl[:, :], fcbt[:, :])
                        nc.vector.tensor_reduce(Mt[:, c:c + 1], pl[:, :], axis=mybir.AxisListType.X,
                                                op=ALU.max)
                        msk = wk.tile([B, CHW], DT.float32, tag="msk", name=f"msk{t_step}_{c}")
                        nc.vector.scalar_tensor_tensor(msk[:, :], pl[:, :], Mt[:, c:c + 1],
                                                       iota_f[:, :], ALU.is_equal, ALU.mult)
                        nc.vector.tensor_reduce(It[:, c:c + 1], msk[:, :], axis=mybir.AxisListType.X,
                                                op=ALU.max)
                        dump = wk.tile([B, CHW], DT.bfloat16, tag="dump", name=f"dump{t_step}_{c}")
                        nc.scalar.activation(dump[:, :], pl[:, :], AF.Exp, bias=zb[:B, :],
                                             scale=1.0, accum_out=sext[:, c:c + 1])
                        lch = wk.tile([B, CHW], DT.bfloat16, tag="lch", bufs=4, name=f"lch{t_step}_{c}")
                        nc.scalar.activation(lch[:, :], pl[:, :], AF.Copy)
                        nc.sync.dma_start(lst_dram[t_step][:, c * CHW:(c + 1) * CHW], lch[:, :])
                    # exp-sum for the step
                    nc.vector.tensor_reduce(stab[:, t_step:t_step + 1], sext[:, :],
                                            axis=mybir.AxisListType.X, op=ALU.add)
                    # global max + its (chunk, idx)
                    gmax = wk.tile([B, 1], DT.float32, tag="gmax", name=f"gmax{t_step}")
                    nc.vector.tensor_reduce(gmax[:, :], Mt[:, :], axis=mybir.AxisListType.X, op=ALU.max)
                    wch = wk.tile([B, NCHUNK], DT.float32, tag="wch", name=f"wch{t_step}")
                    nc.vector.scalar_tensor_tensor(wch[:, :], Mt[:, :], gmax[:, :], ch_f[:, :],
                                                   ALU.is_equal, ALU.mult)
                    wc = wk.tile([B, 1], DT.float32, tag="wc", name=f"wc{t_step}")
                    nc.vector.tensor_reduce(wc[:, :], wch[:, :], axis=mybir.AxisListType.X, op=ALU.max)
                    wij = wk.tile([B, NCHUNK], DT.float32, tag="wij", name=f"wij{t_step}")
                    nc.vector.scalar_tensor_tensor(wij[:, :], Mt[:, :], gmax[:, :], It[:, :],
                                                   ALU.is_equal, ALU.mult)
                    wj = wk.tile([B, 1], DT.float32, tag="wj", name=f"wj{t_step}")
                    nc.vector.tensor_reduce(wj[:, :], wij[:, :], axis=mybir.AxisListType.X, op=ALU.max)
                    # token_global = voff + wc*500 + wj ; pack [val, tok]
                    pack = wk.tile([B, 2], DT.float32, tag="pack", name=f"pack{t_step}")
                    nc.vector.tensor_copy(pack[:, 0:1], gmax[:, :])
                    tok1 = wk.tile([B, 1], DT.float32, tag="tok1", name=f"tok1{t_step}")
                    nc.vector.tensor_scalar_mul(tok1[:, :], wc[:, :], float(CHW))
                    nc.vector.tensor_add(tok1[:, :], tok1[:, :], wj[:, :])
                    nc.vector.tensor_add(pack[:, 1:2], tok1[:, :], vofft[:, :])
                    nc.sync.dma_start(cc_top_in[t_step][:, :], pack[:, :])
                    nc.gpsimd.collective_compute("AllGather", ALU.bypass, ins=[cc_top_in[t_step][:, :]],
                                                 outs=[cc_top_out[t_step][:, :]], replica_groups=rg)
                    topall = wk.tile([B, 2 * NCORE], DT.float32, tag="topall", name=f"topall{t_step}")
                    nc.sync.dma_start(
                        topall[:, :].rearrange("b (c v) -> b c v", v=2),
                        cc_top_out[t_step].rearrange("(c b) v -> b c v", b=B),
                    )
                    t3 = topall[:, :].rearrange("b (c v) -> b c v", v=2)
                    gv = wk.tile([B, 1], DT.float32, tag="gv", name=f"gv{t_step}")
                    nc.vector.tensor_reduce(gv[:, :], t3[:, :, 0], axis=mybir.AxisListType.X, op=ALU.max)
                    wtokf = wk.tile([B, NCORE], DT.float32, tag="wtokf", name=f"wtokf{t_step}")
                    nc.vector.scalar_tensor_tensor(wtokf[:, :], t3[:, :, 0], gv[:, :], t3[:, :, 1],
                                                   ALU.is_equal, ALU.mult)
                    wtok = wk.tile([B, 1], DT.float32, tag="wtok", name=f"wtok{t_step}")
                    nc.vector.tensor_reduce(wtok[:, :], wtokf[:, :], axis=mybir.AxisListType.X, op=ALU.max)
                    wtoki = wk.tile([B, 1], DT.int32, tag="wtoki", name=f"wtoki{t_step}")
                    nc.vector.tensor_copy(wtoki[:, :], wtok[:, :])
                    lemb = wk.tile([B, E], DT.float32, tag="lemb", name=f"lemb{t_step}")
                    nc.gpsimd.indirect_dma_start(
                        out=lemb[:, :], out_offset=None, in_=io["embed"][:, :],
                        in_offset=bass.IndirectOffsetOnAxis(ap=wtoki[:, :1], axis=0),
                    )
                    newlast = []
                    for ch in range(2):
                        pt = ps.tile([128, B], DT.float32, tag="bank", name=f"lt{t_step}_{ch}")
                        nc.tensor.transpose(pt[:, :], lemb[:, ch * 128:(ch + 1) * 128], ident[:B, :B])
                        lt = wk.tile([128, B], DT.float32, tag=f"lastT{ch}", bufs=2, name=f"lastT{t_step}_{ch}")
                        evict(lt[:, :], pt[:, :])
                        newlast.append(lt[:, :])
                    lastT = newlast

                # ================= P7: normalize + write =================
                nc.sync.dma_start(cc_s_in[:, :], stab[:, :])
                nc.gpsimd.collective_compute("AllGather", ALU.bypass, ins=[cc_s_in[:, :]],
                                             outs=[cc_s_out[:, :]], replica_groups=rg)
                sall = wk.tile([B, NCORE * alen], DT.float32, name="sall")
                nc.sync.dma_start(
                    sall[:, :].rearrange("b (t c) -> b t c", c=NCORE),
                    cc_s_out.rearrange("(c b) t -> b t c", b=B),
                )
                stot = wk.tile([B, alen], DT.float32, name="stot")
                nc.vector.tensor_reduce(stot[:, :], sall[:, :].rearrange("b (t c) -> b t c", c=NCORE),
                                        axis=mybir.AxisListType.X, op=ALU.add)
                nlz = wk.tile([B, alen], DT.float32, name="nlz")
                nc.scalar.activation(nlz[:, :], stot[:, :], AF.Ln, bias=zb[:B, :], scale=1.0)
                nc.vector.tensor_scalar_mul(nlz[:, :], nlz[:, :], -1.0)
                out3 = out_logp.rearrange("(b t) v -> b t v", t=alen)
                for t_step in range(n_dec):
                    for c in range(NCHUNK):
                        lc = wk.tile([B, CHW], DT.bfloat16, tag="lc7", bufs=4, name=f"lc7_{t_step}_{c}")
                        nc.sync.dma_start(lc[:, :], lst_dram[t_step][:, c * CHW:(c + 1) * CHW])
                        ot = wk.tile([B, CHW], DT.float32, tag="ot", bufs=4, name=f"ot{t_step}_{c}")
                        if c % 2 == 0:
                            nc.scalar.activation(ot[:, :], lc[:, :], AF.Identity,
                                                 bias=nlz[:, t_step:t_step + 1], scale=1.0)
                        else:
                            nc.vector.tensor_scalar_add(ot[:, :], lc[:, :],
                                                        nlz[:, t_step:t_step + 1])
                        nc.sync.dma_start(out3[:, t_step, c * CHW:(c + 1) * CHW], ot[:, :])

    nc.finalize()
    return nc


def prep_inputs(inputs):
    """Host-side shard/pack. Returns in_maps list for the 8 cores."""
    f32 = np.float32
    emb = np.ascontiguousarray(inputs["embed_w"], dtype=f32)
    packs = {}
    for g in GK:
        wih = np.asarray(inputs[f"{g}_wih"], dtype=f32)
        whh = np.asarray(inputs[f"{g}_whh"], dtype=f32)
        bih = np.asarray(inputs[f"{g}_bih"], dtype=f32)
        bhh = np.asarray(inputs[f"{g}_bhh"], dtype=f32)
        brz = 0.5 * (bih[:512] + bhh[:512])
        packs[f"{g}_wihT"] = np.ascontiguousarray(wih.T)
        packs[f"{g}_whhT"] = np.ascontiguousarray(whh.T)
        packs[f"{g}_hbrz"] = np.ascontiguousarray(brz.reshape(4, 128).T)
        packs[f"{g}_bin"] = np.ascontiguousarray(bih[512:768].reshape(2, 128).T)
        packs[f"{g}_bhn"] = np.ascontiguousarray(bhh[512:768].reshape(2, 128).T)
    packs["g1T"] = np.ascontiguousarray(np.asarray(inputs["gate_w1"], f32).T)
    packs["g2T"] = np.ascontiguousarray(np.asarray(inputs["gate_w2"], f32).T)
    packs["gb1"] = np.ascontiguousarray(np.asarray(inputs["gate_b1"], f32).reshape(2, 128).T)
    packs["gb2h"] = (0.5 * np.asarray(inputs["gate_b2"], f32)).reshape(1, 1)
    fcwT = np.ascontiguousarray(np.asarray(inputs["fc_w"], f32).T)
    fcb = np.asarray(inputs["fc_b"], f32)
    fcb_nonzero = bool(np.any(fcb != 0))
    last0T = np.ascontiguousarray(np.tile(emb[SEQBEGIN][:, None], (1, B)))
    allfacts = np.asarray(inputs["allfacts"], np.int32)
    questions = np.asarray(inputs["questions"], np.int32)

    in_maps = []
    for j in range(NCORE):
        m = dict(packs)
        m["embed"] = emb
        m["fcwT"] = np.ascontiguousarray(fcwT[:, j * VS:(j + 1) * VS])
        if fcb_nonzero:
            m["fcb"] = np.ascontiguousarray(np.tile(fcb[None, j * VS:(j + 1) * VS], (B, 1)))
        m["last0T"] = last0T
        m["voff"] = np.full((B, 1), j * VS, f32)
        m["facts_idx"] = np.ascontiguousarray(
            allfacts[j * BB:(j + 1) * BB].reshape(NSEQ, FL).T.reshape(-1, 1))
        m["q_idx"] = np.ascontiguousarray(
            questions[j * BB:(j + 1) * BB].reshape(BB, QL).T.reshape(-1, 1))
        in_maps.append(m)
    return in_maps, fcb_nonzero


_CACHE = {}


def kernel(**inputs):
    alen = int(inputs["alen"])
    in_maps, fcb_nonzero = prep_inputs(inputs)
    key = (alen, fcb_nonzero)
    if key not in _CACHE:
        _CACHE[key] = build_nc(alen, fcb_nonzero)
    nc = _CACHE[key]
    res = run_bass_kernel_spmd(nc, in_maps, core_ids=list(range(NCORE)))
    out = np.concatenate([res.results[j]["out_logp"] for j in range(NCORE)], axis=1)
    return out.astype(np.float32)


if __name__ == "__main__":
    data = dict(np.load("/root/problem/inputs_cpu.npz"))
    data["alen"] = 8
    out = kernel(**data)
    exp = np.load("/root/problem/expected_cpu.npy")
    d = np.abs(out - exp)
    print("maxabs", d.max(), "relnorm", np.linalg.norm(out - exp) / np.linalg.norm(exp))



# revision 7
# speedup vs baseline: 1.9240x; 1.9240x over previous
"""DMN forward on 8 Trainium2 NeuronCores (Bass/Tile), bf16 matmul build.

Sharding: batch rows 8/core for fact+question encoding and episodic memory
(core j owns batch rows 8j..8j+7 and their 160 fact sequences); decode GRU
replicated on all cores, fc/log-softmax vocab-sharded 4000 columns/core, with
one small AllGather per decode step carrying (max, argmax-token, expsum) for
the greedy feedback and the log-softmax normalizer.

All matmuls run in bf16 (PE single-pass, 1 cycle/row) with fp32 PSUM
accumulation; GRU non-linearities use the native Sigmoid activation in the
encoder table and the tanh form in decode (so Exp shares the same table).
Each GRU step packs all six gate blocks into a single PSUM bank as column
groups, so the elementwise tail is ~9 fused ops. The fact-token projection is
hoisted, held in SBUF (bf16), and its biases folded in at PSUM-evict time.
The log-softmax normalizer uses a bitcast fast-log plus one exp-Newton step
instead of Ln, keeping the scalar engine on one activation table all decode.
"""

import os
import numpy as np

import concourse.bass as bass
import concourse.bacc as bacc
import concourse.mybir as mybir
from concourse.tile import TileContext
from concourse.bass_utils import run_bass_kernel_spmd
from concourse.masks import make_identity

AF = mybir.ActivationFunctionType
ALU = mybir.AluOpType
DT = mybir.dt
BF = DT.bfloat16
F32 = DT.float32

V, E, H = 32000, 256, 256
B, NF, FL, QL = 64, 20, 32, 16
N_EPISODE = 3
SEQBEGIN = 1
NCORE = 8
BB = B // NCORE            # batch rows per core = 8
NSEQ = BB * NF             # fact seqs per core = 160
NTOK = NSEQ * FL           # fact tokens per core = 5120
VS = V // NCORE            # vocab shard = 4000
NCHUNK = 8
CHW = VS // NCHUNK         # 500
FH = NSEQ // 2             # fact chain width = 80

GK = {"ig": E, "qg": E, "att": H, "mem": H, "ans": 2 * H}

# fast-log constants: ln(x) ~ (bitcast_i32(x) - LOGB) * LOGK, refined by one
# Newton step y <- y + (x*exp(-y) - 1)
LOGK = float(np.log(2.0) / (1 << 23))
LOGB = 1064866805.0


def build_nc(alen, fcb_nonzero):
    nc = bacc.Bacc("TRN2", num_devices=NCORE)

    def dram_in(name, shape, dtype=F32):
        return nc.dram_tensor(name, list(shape), dtype, kind="ExternalInput")

    io = {}
    io["facts_idx"] = dram_in("facts_idx", [NTOK, 1], DT.int32)
    io["q_idx"] = dram_in("q_idx", [BB * QL, 1], DT.int32)
    io["embed"] = dram_in("embed", [V, E], BF)
    io["fcwT"] = dram_in("fcwT", [E, VS], BF)
    io["l0"] = dram_in("l0", [128, 2 * B], BF)
    io["voff"] = dram_in("voff", [B, 1])
    if fcb_nonzero:
        io["fcb"] = dram_in("fcb", [B, VS])
    for g, kin in GK.items():
        io[f"{g}_wihT"] = dram_in(f"{g}_wihT", [kin, 3 * H], BF)
        io[f"{g}_whhT"] = dram_in(f"{g}_whhT", [H, 3 * H], BF)
        io[f"{g}_brz6"] = dram_in(f"{g}_brz6", [128, 6])
        io[f"{g}_bhn"] = dram_in(f"{g}_bhn", [128, 2])
    io["g1T"] = dram_in("g1T", [4 * H, H], BF)
    io["g2T"] = dram_in("g2T", [H, 1], BF)
    io["gb1"] = dram_in("gb1", [128, 2])
    io["gb2"] = dram_in("gb2", [1, 1])
    io["memrz"] = dram_in("memrz", [128, 4 * BB])
    io["membin"] = dram_in("membin", [128, 2 * BB])

    out_logp = nc.dram_tensor("out_logp", [B * alen, VS], F32, kind="ExternalOutput")

    cc_enc_in = nc.dram_tensor("cc_enc_in", [BB, 2 * H], F32, kind="Internal")
    cc_enc_out = nc.dram_tensor("cc_enc_out", [B, 2 * H], F32, kind="Internal", addr_space="Shared")
    cc_top_in = [nc.dram_tensor(f"cc_top_in{t}", [B, 4], F32, kind="Internal") for t in range(alen)]
    cc_top_out = [nc.dram_tensor(f"cc_top_out{t}", [NCORE * B, 4], F32, kind="Internal",
                                 addr_space="Shared") for t in range(alen)]
    rg = [list(range(NCORE))]

    dbg = int(os.environ.get("K_DEBUG_STEPS", "0"))
    n_fl = dbg or FL
    n_ql = dbg or QL
    n_nf = dbg or NF
    n_ep = 1 if dbg else N_EPISODE
    n_dec = min(alen, dbg) if dbg else alen

    with TileContext(nc) as tc:
        with tc.tile_pool(name="shared", bufs=1) as shp, \
             tc.tile_pool(name="state", bufs=1) as st, \
             tc.tile_pool(name="work", bufs=3) as wk, \
             tc.tile_pool(name="ps", bufs=8, space="PSUM") as ps:

            identB = shp.tile([128, 128], BF)
            make_identity(nc, identB[:, :])
            identF = shp.tile([128, 128], F32)
            make_identity(nc, identF[:, :])

            W = {}

            def load_gate(pool, g):
                kin = GK[g]
                xw = []
                for k in range(kin // 128):
                    t = pool.tile([128, 3 * H], BF, name=f"{g}xw{k}")
                    nc.sync.dma_start(t[:, :], io[f"{g}_wihT"][k * 128:(k + 1) * 128, :])
                    xw.append(t)
                hw = []
                for k in range(2):
                    t = pool.tile([128, 3 * H], BF, name=f"{g}hw{k}")
                    nc.sync.dma_start(t[:, :], io[f"{g}_whhT"][k * 128:(k + 1) * 128, :])
                    hw.append(t)
                brz = pool.tile([128, 6], F32, name=f"{g}brz")
                nc.sync.dma_start(brz[:, :], io[f"{g}_brz6"][:, :])
                bhn = pool.tile([128, 2], F32, name=f"{g}bhn")
                nc.sync.dma_start(bhn[:, :], io[f"{g}_bhn"][:, :])
                W[g] = (xw, hw, brz, bhn)

            evict_rr = [0]

            def evict(dst_ap, src_ap, bias=None):
                if bias is None:
                    if evict_rr[0] % 2 == 0:
                        nc.vector.tensor_copy(dst_ap, src_ap)
                    else:
                        nc.scalar.activation(dst_ap, src_ap, AF.Copy)
                else:
                    if evict_rr[0] % 2 == 0:
                        nc.vector.tensor_scalar(dst_ap, src_ap, bias, None, ALU.add)
                    else:
                        nc.scalar.activation(dst_ap, src_ap, AF.Identity, bias=bias)
                evict_rr[0] += 1

            # ---- one fused GRU step -------------------------------------
            # bank: psum [128, 6F] (m-groups r0 r1 z0 z1 n0 n1, F cols each)
            # h: [128, 2F] bf16 (hidden half k at cols kF).  Emits matmuls +
            # elementwise; returns h' [128, 2F] bf16 (or e' for att form).
            def gru_mms(bank, g, h_ap, F, x_rhs=None, xw_override=None):
                xw, hw, _, _ = W[g]
                if xw_override is not None:
                    xw = xw_override
                for m in range(6):
                    dst = bank[:, m * F:(m + 1) * F]
                    first = True
                    if x_rhs is not None:
                        for k in range(len(x_rhs)):
                            nc.tensor.matmul(dst, xw[k][:, m * 128:(m + 1) * 128],
                                             x_rhs[k], start=first, stop=False)
                            first = False
                    for k in range(2):
                        last = (k == 1)
                        nc.tensor.matmul(dst, hw[k][:, m * 128:(m + 1) * 128],
                                         h_ap[:, k * F:(k + 1) * F],
                                         start=first, stop=last)
                        first = False

            def gru_ew(bank, g, h_ap, F, gi_rz, gi_n, name, sig_direct=True,
                       att=None):
                """gi_rz: AP [128, 4F]-size (x-proj + all r/z biases);
                gi_n: AP [128, 2F]-size (x-proj n + bih_n).
                att: None for plain GRU; (e_ap, negg_ap) for episode form."""
                _, _, _, bhn = W[g]
                trz = wk.tile([128, 4 * F], BF, tag=f"trz{F}{name[0]}", bufs=2,
                              name=f"{name}trz")
                nc.vector.tensor_add(trz[:, :].rearrange("p (m f) -> p m f", f=F),
                                     bank[:, 0:4 * F].rearrange("p (m f) -> p m f", f=F),
                                     gi_rz)
                rz = wk.tile([128, 4 * F], BF, tag=f"rz{F}{name[0]}", bufs=2,
                             name=f"{name}rz")
                if sig_direct:
                    nc.scalar.activation(rz[:, :], trz[:, :], AF.Sigmoid)
                else:
                    th = wk.tile([128, 4 * F], BF, tag=f"th{F}{name[0]}", bufs=2,
                                 name=f"{name}th")
                    nc.scalar.activation(th[:, :], trz[:, :], AF.Tanh, scale=0.5)
                    nc.vector.tensor_scalar(rz[:, :], th[:, :], 0.5, 0.5,
                                            ALU.mult, ALU.add)
                y = wk.tile([128, 2 * F], BF, tag=f"y{F}{name[0]}", bufs=2,
                            name=f"{name}y")
                for h in range(2):
                    nc.vector.scalar_tensor_tensor(
                        y[:, h * F:(h + 1) * F], bank[:, (4 + h) * F:(5 + h) * F],
                        bhn[:, h:h + 1], rz[:, h * F:(h + 1) * F], ALU.add, ALU.mult)
                u = wk.tile([128, 2 * F], BF, tag=f"u{F}{name[0]}", bufs=2,
                            name=f"{name}u")
                nc.vector.tensor_add(u[:, :].rearrange("p (m f) -> p m f", f=F),
                                     y[:, :].rearrange("p (m f) -> p m f", f=F), gi_n)
                n = wk.tile([128, 2 * F], BF, tag=f"n{F}{name[0]}", bufs=2,
                            name=f"{name}n")
                nc.scalar.activation(n[:, :], u[:, :], AF.Tanh)
                if att is None:
                    # h' = n + z * (h - n)
                    d = wk.tile([128, 2 * F], BF, tag=f"d{F}{name[0]}", bufs=2,
                                name=f"{name}d")
                    nc.vector.tensor_sub(d[:, :], h_ap, n[:, :])
                    w2 = wk.tile([128, 2 * F], BF, tag=f"w{F}{name[0]}", bufs=2,
                                 name=f"{name}w")
                    nc.vector.tensor_mul(w2[:, :], rz[:, 2 * F:4 * F], d[:, :])
                    hn = wk.tile([128, 2 * F], BF, tag=f"hn{F}{name[0]}", bufs=2,
                                 name=f"{name}hn")
                    nc.vector.tensor_add(hn[:, :], n[:, :], w2[:, :])
                    return hn
                else:
                    # e' = e + g*(1-z)*(n - e);  w1 = (z - 1)*(-g)
                    e_ap, negg = att
                    d = wk.tile([128, 2 * F], BF, tag=f"d{F}{name[0]}", bufs=2,
                                name=f"{name}d")
                    nc.vector.tensor_sub(d[:, :], n[:, :], e_ap)
                    w1 = wk.tile([128, 2 * F], BF, tag=f"w1{F}{name[0]}", bufs=2,
                                 name=f"{name}w1")
                    nc.vector.scalar_tensor_tensor(
                        w1[:, :].rearrange("p (m f) -> p m f", f=F),
                        rz[:, 2 * F:4 * F].rearrange("p (m f) -> p m f", f=F),
                        1.0, negg, ALU.subtract, ALU.mult)
                    p2 = wk.tile([128, 2 * F], BF, tag=f"p2{F}{name[0]}", bufs=2,
                                 name=f"{name}p2")
                    nc.vector.tensor_mul(p2[:, :], w1[:, :], d[:, :])
                    en = wk.tile([128, 2 * F], BF, tag=f"hn{F}{name[0]}", bufs=2,
                                 name=f"{name}en")
                    nc.vector.tensor_add(en[:, :], e_ap, p2[:, :])
                    return en

            # ================= P1: gather + x-projection =================
            with tc.tile_pool(name="fpool", bufs=1) as fp:
                load_gate(fp, "ig")
                load_gate(fp, "qg")
                XT = [fp.tile([128, NTOK], BF, name=f"XT{k}") for k in range(2)]
                fidx = fp.tile([128, NTOK // 128], DT.int32, name="fidx")
                nc.sync.dma_start(fidx[:, :], io["facts_idx"].rearrange("(b a) o -> a (b o)", a=128))
                for i in range(NTOK // 128):
                    gt = wk.tile([128, E], BF, tag="fgat", bufs=4, name=f"fg{i}")
                    nc.gpsimd.indirect_dma_start(
                        out=gt[:, :], out_offset=None, in_=io["embed"][:, :],
                        in_offset=bass.IndirectOffsetOnAxis(ap=fidx[:, i:i + 1], axis=0),
                    )
                    for ch in range(2):
                        pt = ps.tile([128, 128], BF, tag="bank", name=f"ftp{i}_{ch}")
                        nc.tensor.transpose(pt[:, :], gt[:, ch * 128:(ch + 1) * 128], identB[:, :])
                        evict(XT[ch][:, i * 128:(i + 1) * 128], pt[:, :])

                # hoisted fact x-projection, SBUF-resident bf16, biases folded
                gi = fp.tile([128, 6 * NTOK], BF, name="gi")
                gi3 = gi[:, :].rearrange("p (m t) -> p m t", t=NTOK)
                NCH = NTOK // 512
                ig_brz = W["ig"][2]
                for m in range(6):
                    for c in range(NCH):
                        pp = ps.tile([128, 512], F32, tag="bank", name=f"xp{m}_{c}")
                        for k in range(2):
                            nc.tensor.matmul(pp[:, :], W["ig"][0][k][:, m * 128:(m + 1) * 128],
                                             XT[k][:, c * 512:(c + 1) * 512],
                                             start=(k == 0), stop=(k == 1))
                        evict(gi3[:, m, c * 512:(c + 1) * 512], pp[:, :],
                              bias=ig_brz[:, m:m + 1])

                # question gather + x-projection
                qidx = wk.tile([128, 1], DT.int32, name="qidx")
                nc.sync.dma_start(qidx[:, :], io["q_idx"][:, :])
                qg_t = wk.tile([128, E], BF, tag="fgat", bufs=4, name="qgat")
                nc.gpsimd.indirect_dma_start(
                    out=qg_t[:, :], out_offset=None, in_=io["embed"][:, :],
                    in_offset=bass.IndirectOffsetOnAxis(ap=qidx[:, :1], axis=0),
                )
                XQ = fp.tile([128, 2 * BB * QL], BF, name="XQ")
                for ch in range(2):
                    pt = ps.tile([128, 128], BF, tag="bank", name=f"qtp{ch}")
                    nc.tensor.transpose(pt[:, :], qg_t[:, ch * 128:(ch + 1) * 128], identB[:, :])
                    evict(XQ[:, ch * 128:(ch + 1) * 128], pt[:, :])
                giq = fp.tile([128, 6 * BB * QL], BF, name="giq")
                giq3 = giq[:, :].rearrange("p (m t) -> p m t", t=BB * QL)
                qg_brz = W["qg"][2]
                for m in range(6):
                    pp = ps.tile([128, BB * QL], F32, tag="bank", name=f"qxp{m}")
                    for k in range(2):
                        nc.tensor.matmul(pp[:, :], W["qg"][0][k][:, m * 128:(m + 1) * 128],
                                         XQ[:, k * 128:(k + 1) * 128], start=(k == 0), stop=(k == 1))
                    evict(giq3[:, m, :], pp[:, :], bias=qg_brz[:, m:m + 1])

                # ============ P2+P3: fact GRU (2 chains) + question GRU ====
                hA = wk.tile([128, NSEQ], BF, tag="hA", bufs=2, name="hA0")
                hB = wk.tile([128, NSEQ], BF, tag="hB", bufs=2, name="hB0")
                hq = wk.tile([128, 2 * BB], BF, tag="hq", bufs=2, name="hq0")
                nc.vector.memset(hA[:, :], 0.0)
                nc.vector.memset(hB[:, :], 0.0)
                nc.vector.memset(hq[:, :], 0.0)
                for t in range(n_fl):
                    bankA = ps.tile([128, 6 * FH], F32, tag="bank", name=f"bA{t}")
                    bankB = ps.tile([128, 6 * FH], F32, tag="bank", name=f"bB{t}")
                    gru_mms(bankA, "ig", hA[:, :], FH)
                    gru_mms(bankB, "ig", hB[:, :], FH)
                    do_q = (t % 2 == 0) and (t // 2 < n_ql)
                    if do_q:
                        tq = t // 2
                        bankQ = ps.tile([128, 6 * BB], F32, tag="bank", name=f"bQ{tq}")
                        gru_mms(bankQ, "qg", hq[:, :], BB)
                    t0 = t * NSEQ
                    hA = gru_ew(bankA, "ig", hA[:, :], FH,
                                gi3[:, 0:4, t0:t0 + FH], gi3[:, 4:6, t0:t0 + FH],
                                name=f"A{t}_")
                    hB = gru_ew(bankB, "ig", hB[:, :], FH,
                                gi3[:, 0:4, t0 + FH:t0 + NSEQ],
                                gi3[:, 4:6, t0 + FH:t0 + NSEQ], name=f"B{t}_")
                    if do_q:
                        q0 = tq * BB
                        hq = gru_ew(bankQ, "qg", hq[:, :], BB,
                                    giq3[:, 0:4, q0:q0 + BB], giq3[:, 4:6, q0:q0 + BB],
                                    name=f"Q{tq}_")

                # persist enc_facts [128, 2, NSEQ] and enc_q [128, 2*BB]
                encf = st.tile([128, 2 * NSEQ], BF, name="encf")
                for h in range(2):
                    nc.vector.tensor_copy(encf[:, h * NSEQ:h * NSEQ + FH],
                                          hA[:, h * FH:(h + 1) * FH])
                    nc.vector.tensor_copy(encf[:, h * NSEQ + FH:(h + 1) * NSEQ],
                                          hB[:, h * FH:(h + 1) * FH])
                hqF = st.tile([128, 2 * BB], BF, name="hqF")
                nc.vector.tensor_copy(hqF[:, :], hq[:, :])

            # ================= P4: episodes =================
            with tc.tile_pool(name="epool", bufs=1) as epl:
                load_gate(epl, "att")
                load_gate(epl, "mem")
                g1T = []
                for k in range(8):
                    tt = epl.tile([128, H], BF, name=f"g1T{k}")
                    nc.sync.dma_start(tt[:, :], io["g1T"][k * 128:(k + 1) * 128, :])
                    g1T.append(tt)
                g2T = []
                for k in range(2):
                    tt = epl.tile([128, 1], BF, name=f"g2T{k}")
                    nc.sync.dma_start(tt[:, :], io["g2T"][k * 128:(k + 1) * 128, :])
                    g2T.append(tt)
                gb1 = epl.tile([128, 2], F32)
                nc.sync.dma_start(gb1[:, :], io["gb1"][:, :])
                gb2 = epl.tile([1, 1], F32)
                nc.sync.dma_start(gb2[:, :], io["gb2"][:, :])
                memrz = epl.tile([128, 4 * BB], F32)
                nc.sync.dma_start(memrz[:, :], io["memrz"][:, :])
                membin = epl.tile([128, 2 * BB], F32)
                nc.sync.dma_start(membin[:, :], io["membin"][:, :])

                # att x-projection of enc_facts (+ biases)
                giaP = epl.tile([128, 6 * NSEQ], BF, name="giaP")
                gia3 = giaP[:, :].rearrange("p (m t) -> p m t", t=NSEQ)
                att_brz = W["att"][2]
                for m in range(6):
                    pp = ps.tile([128, NSEQ], F32, tag="bank", name=f"axp{m}")
                    for k in range(2):
                        nc.tensor.matmul(pp[:, :], W["att"][0][k][:, m * 128:(m + 1) * 128],
                                         encf[:, k * NSEQ:(k + 1) * NSEQ],
                                         start=(k == 0), stop=(k == 1))
                    evict(gia3[:, m, :], pp[:, :], bias=att_brz[:, m:m + 1])
                gia4 = giaP[:, :].rearrange("p (m b i) -> p m b i", m=6, i=NF)

                memT = wk.tile([128, 2 * BB], BF, tag="memT", bufs=2, name="memT0")
                nc.vector.tensor_copy(memT[:, :], hqF[:, :])
                encf3 = [encf[:, k * NSEQ:(k + 1) * NSEQ].rearrange("p (r i) -> p r i", i=NF)
                         for k in range(2)]

                for ep in range(n_ep):
                    # gate features z = [f*q, f*m, |f-q|, |f-m|]
                    ZT = [wk.tile([128, NSEQ], BF, tag=f"zt{x}", bufs=1, name=f"ZT{ep}_{x}")
                          for x in range(8)]
                    for h in range(2):
                        qb = hqF[:, h * BB:(h + 1) * BB].to_broadcast([128, BB, NF])
                        mb = memT[:, h * BB:(h + 1) * BB].to_broadcast([128, BB, NF])
                        z3 = [ZT[x][:, :].rearrange("p (r i) -> p r i", i=NF) for x in range(8)]
                        nc.vector.tensor_mul(z3[0 + h], encf3[h], qb)
                        nc.vector.tensor_mul(z3[2 + h], encf3[h], mb)
                        dq = wk.tile([128, NSEQ], F32, tag="dq", bufs=2, name=f"dq{ep}_{h}")
                        nc.vector.tensor_sub(dq[:, :].rearrange("p (r i) -> p r i", i=NF),
                                             encf3[h], qb)
                        nc.scalar.activation(ZT[4 + h][:, :], dq[:, :], AF.Abs)
                        dm = wk.tile([128, NSEQ], F32, tag="dm", bufs=2, name=f"dm{ep}_{h}")
                        nc.vector.tensor_sub(dm[:, :].rearrange("p (r i) -> p r i", i=NF),
                                             encf3[h], mb)
                        nc.scalar.activation(ZT[6 + h][:, :], dm[:, :], AF.Abs)
                    p1T = []
                    for m in range(2):
                        pp = ps.tile([128, NSEQ], F32, tag="bank", name=f"p1{ep}_{m}")
                        for k in range(8):
                            nc.tensor.matmul(pp[:, :], g1T[k][:, m * 128:(m + 1) * 128],
                                             ZT[k][:, :], start=(k == 0), stop=(k == 7))
                        t1 = wk.tile([128, NSEQ], BF, tag="p1s", bufs=2, name=f"p1s{ep}_{m}")
                        nc.scalar.activation(t1[:, :], pp[:, :], AF.Tanh, bias=gb1[:, m:m + 1])
                        p1T.append(t1)
                    pgp = ps.tile([1, NSEQ], F32, tag="bank", name=f"pg{ep}")
                    for k in range(2):
                        nc.tensor.matmul(pgp[:, :], g2T[k][:, :], p1T[k][:, :],
                                         start=(k == 0), stop=(k == 1))
                    g_row = wk.tile([1, NSEQ], F32, tag="grow", bufs=1, name=f"grow{ep}")
                    nc.scalar.activation(g_row[:, :], pgp[:, :], AF.Sigmoid, bias=gb2[:1, :1])
                    ngrow = wk.tile([1, NSEQ], F32, tag="ngrow", bufs=1, name=f"ngrow{ep}")
                    nc.vector.tensor_scalar_mul(ngrow[:, :], g_row[:, :], -1.0)
                    negG = wk.tile([128, 2 * NSEQ], F32, tag="negG", bufs=1, name=f"negG{ep}")
                    nc.gpsimd.partition_broadcast(negG[:, 0:NSEQ], ngrow[:, :])
                    nc.vector.tensor_copy(negG[:, NSEQ:2 * NSEQ], negG[:, 0:NSEQ])
                    negG4 = negG[:, :].rearrange("p (h b i) -> p h b i", h=2, i=NF)

                    eT = wk.tile([128, 2 * BB], BF, tag="eT", bufs=2, name=f"eT{ep}")
                    nc.vector.memset(eT[:, :], 0.0)
                    for i in range(n_nf):
                        bank = ps.tile([128, 6 * BB], F32, tag="bank",
                                       name=f"ab{ep}_{i}")
                        gru_mms(bank, "att", eT[:, :], BB)
                        eT = gru_ew(bank, "att", eT[:, :], BB,
                                    gia4[:, 0:4, :, i], gia4[:, 4:6, :, i],
                                    name=f"e{ep}_{i}_", att=(eT[:, :], negG4[:, :, :, i]))
                    # memory GRU step (x = eT)
                    bank = ps.tile([128, 6 * BB], F32, tag="bank", name=f"mb{ep}")
                    gru_mms(bank, "mem", memT[:, :], BB,
                            x_rhs=[eT[:, k * BB:(k + 1) * BB] for k in range(2)])
                    memT = gru_ew(bank, "mem", memT[:, :], BB,
                                  memrz[:, :].rearrange("p (m f) -> p m f", f=BB),
                                  membin[:, :].rearrange("p (m f) -> p m f", f=BB),
                                  name=f"m{ep}_")

                memF = st.tile([128, 2 * BB], BF, name="memF")
                nc.vector.tensor_copy(memF[:, :], memT[:, :])

            # ================= P5: all-gather mem|enc_q =================
            encrow = wk.tile([BB, 2 * H], F32, name="encrow")
            for h in range(2):
                pt = ps.tile([BB, 128], BF, tag="bank", name=f"egm{h}")
                nc.tensor.transpose(pt[:, :], memF[:, h * BB:(h + 1) * BB], identB[:, :])
                evict(encrow[:, h * 128:(h + 1) * 128], pt[:, :])
                pt2 = ps.tile([BB, 128], BF, tag="bank", name=f"egq{h}")
                nc.tensor.transpose(pt2[:, :], hqF[:, h * BB:(h + 1) * BB], identB[:, :])
                evict(encrow[:, 256 + h * 128:256 + (h + 1) * 128], pt2[:, :])
            nc.sync.dma_start(cc_enc_in[:, :], encrow[:, :])
            nc.gpsimd.collective_compute("AllGather", ALU.bypass, ins=[cc_enc_in[:, :]],
                                         outs=[cc_enc_out[:, :]], replica_groups=rg)
            enc_all = wk.tile([B, 2 * H], F32, name="enc_all")
            nc.sync.dma_start(enc_all[:, :], cc_enc_out[:, :])

            # ================= P6: decode =================
            with tc.tile_pool(name="dpool", bufs=1) as dp:
                load_gate(dp, "ans")
                fcwT = []
                for k in range(2):
                    tt = dp.tile([128, VS], BF, name=f"fcwT{k}")
                    nc.sync.dma_start(tt[:, :], io["fcwT"][k * 128:(k + 1) * 128, :])
                    fcwT.append(tt)
                l0 = dp.tile([128, 2 * B], BF, name="l0")
                nc.sync.dma_start(l0[:, :], io["l0"][:, :])
                vofft = dp.tile([B, 1], F32)
                nc.sync.dma_start(vofft[:, :], io["voff"][:, :])
                iota_i = dp.tile([B, VS], DT.int32)
                nc.gpsimd.iota(iota_i[:, :], pattern=[[1, VS]], base=0, channel_multiplier=0)
                iotaG = dp.tile([B, VS], F32)
                nc.vector.tensor_copy(iotaG[:, :], iota_i[:, :])

                # transposed mem/enc_q for all 64 rows
                hid = wk.tile([128, 2 * B], BF, tag="hid", bufs=2, name="hid0")
                qA = dp.tile([128, 2 * B], BF, name="qA")
                for h in range(2):
                    pt = ps.tile([128, B], F32, tag="bank", name=f"tmA{h}")
                    nc.tensor.transpose(pt[:, :], enc_all[:, h * 128:(h + 1) * 128],
                                        identF[:B, :B])
                    evict(hid[:, h * B:(h + 1) * B], pt[:, :])
                    pt2 = ps.tile([128, B], F32, tag="bank", name=f"tqA{h}")
                    nc.tensor.transpose(pt2[:, :], enc_all[:, 256 + h * 128:256 + (h + 1) * 128],
                                        identF[:B, :B])
                    evict(qA[:, h * B:(h + 1) * B], pt2[:, :])

                # enc_q half of the ans input projection (+ all r/z/n biases)
                giq6 = dp.tile([128, 6 * B], BF, name="giq6")
                giq63 = giq6[:, :].rearrange("p (m t) -> p m t", t=B)
                ans_xw = W["ans"][0]
                ans_brz = W["ans"][2]
                for m in range(6):
                    pp = ps.tile([128, B], F32, tag="bank", name=f"dxp{m}")
                    for k in range(2):
                        nc.tensor.matmul(pp[:, :], ans_xw[2 + k][:, m * 128:(m + 1) * 128],
                                         qA[:, k * B:(k + 1) * B], start=(k == 0), stop=(k == 1))
                    evict(giq63[:, m, :], pp[:, :], bias=ans_brz[:, m:m + 1])

                lastT = l0
                prev_store = None
                prev_nlz = None
                out3 = out_logp.rearrange("(b t) v -> b t v", t=alen)

                def write_step(ts, store, nlzneg):
                    for vv in range(2):
                        sl = store[:, vv * (VS // 2):(vv + 1) * (VS // 2)]
                        ot = dp.tile([B, VS // 2], F32, tag=f"ot{vv}", bufs=2,
                                     name=f"ot{ts}_{vv}")
                        if vv == 0:
                            nc.vector.tensor_scalar(ot[:, :], sl, nlzneg, None, ALU.add)
                        else:
                            nc.scalar.activation(ot[:, :], sl, AF.Identity, bias=nlzneg)
                        nc.sync.dma_start(out3[:, ts, vv * (VS // 2):(vv + 1) * (VS // 2)],
                                          ot[:, :])

                for ts in range(n_dec):
                    # --- ans GRU step (x = [last, enc_q]) ---
                    bank = ps.tile([128, 6 * B], F32, tag="bank", name=f"db{ts}")
                    gru_mms(bank, "ans", hid[:, :], B,
                            x_rhs=[lastT[:, k * B:(k + 1) * B] for k in range(2)],
                            xw_override=ans_xw[:2])
                    hid = gru_ew(bank, "ans", hid[:, :], B,
                                 giq63[:, 0:4, :], giq63[:, 4:6, :],
                                 name=f"a{ts}_", sig_direct=False)

                    # --- fc + scan ---
                    store = dp.tile([B, VS], F32, tag="lst", bufs=2, name=f"lst{ts}")
                    Mt = wk.tile([B, NCHUNK], F32, tag="Mt", bufs=2, name=f"Mt{ts}")
                    It = wk.tile([B, NCHUNK], F32, tag="It", bufs=2, name=f"It{ts}")
                    sx = wk.tile([B, NCHUNK], F32, tag="sx", bufs=2, name=f"sx{ts}")
                    for c in range(NCHUNK):
                        pl = ps.tile([B, CHW], F32, tag="bank", name=f"pl{ts}_{c}")
                        for k in range(2):
                            nc.tensor.matmul(pl[:, :], hid[:, k * B:(k + 1) * B],
                                             fcwT[k][:, c * CHW:(c + 1) * CHW],
                                             start=(k == 0), stop=(k == 1))
                        if fcb_nonzero:
                            fcbt = wk.tile([B, CHW], F32, tag="fcbt", bufs=2,
                                           name=f"fcbt{ts}_{c}")
                            nc.sync.dma_start(fcbt[:, :], io["fcb"][:, c * CHW:(c + 1) * CHW])
                            nc.vector.tensor_add(pl[:, :], pl[:, :], fcbt[:, :])
                        nc.scalar.activation(store[:, c * CHW:(c + 1) * CHW],
                                             pl[:, :], AF.Copy)
                        dump = dp.tile([B, CHW], BF, tag="dump", bufs=4, name=f"dump{ts}_{c}")
                        nc.scalar.activation(dump[:, :], pl[:, :], AF.Exp,
                                             accum_out=sx[:, c:c + 1])
                        nc.vector.tensor_reduce(Mt[:, c:c + 1],
                                                store[:, c * CHW:(c + 1) * CHW],
                                                axis=mybir.AxisListType.X, op=ALU.max)
                        msk = dp.tile([B, CHW], F32, tag="msk", bufs=2, name=f"msk{ts}_{c}")
                        nc.vector.scalar_tensor_tensor(
                            msk[:, :], store[:, c * CHW:(c + 1) * CHW], Mt[:, c:c + 1],
                            iotaG[:, c * CHW:(c + 1) * CHW], ALU.is_equal, ALU.mult,
                            accum_out=It[:, c:c + 1])
                    # --- local argmax resolve + expsum ---
                    gmax = wk.tile([B, 1], F32, tag="gmax", bufs=2, name=f"gmax{ts}")
                    nc.vector.tensor_reduce(gmax[:, :], Mt[:, :], axis=mybir.AxisListType.X,
                                            op=ALU.max)
                    wsel = wk.tile([B, NCHUNK], F32, tag="wsel", bufs=2, name=f"wsel{ts}")
                    nc.vector.scalar_tensor_tensor(wsel[:, :], Mt[:, :], gmax[:, :],
                                                   It[:, :], ALU.is_equal, ALU.mult)
                    tokf = wk.tile([B, 1], F32, tag="tokf", bufs=2, name=f"tokf{ts}")
                    nc.vector.tensor_reduce(tokf[:, :], wsel[:, :], axis=mybir.AxisListType.X,
                                            op=ALU.max)
                    sxs = wk.tile([B, 1], F32, tag="sxs", bufs=2, name=f"sxs{ts}")
                    nc.vector.tensor_reduce(sxs[:, :], sx[:, :], axis=mybir.AxisListType.X,
                                            op=ALU.add)
                    pack = wk.tile([B, 4], F32, tag="pack", bufs=2, name=f"pack{ts}")
                    nc.vector.tensor_copy(pack[:, 0:1], gmax[:, :])
                    nc.vector.tensor_add(pack[:, 1:2], tokf[:, :], vofft[:, :])
                    nc.vector.tensor_copy(pack[:, 2:3], sxs[:, :])
                    nc.vector.memset(pack[:, 3:4], 0.0)
                    nc.sync.dma_start(cc_top_in[ts][:, :], pack[:, :])
                    nc.gpsimd.collective_compute("AllGather", ALU.bypass,
                                                 ins=[cc_top_in[ts][:, :]],
                                                 outs=[cc_top_out[ts][:, :]], replica_groups=rg)
                    topall = wk.tile([B, 4 * NCORE], F32, tag="topall", bufs=2,
                                     name=f"topall{ts}")
                    nc.sync.dma_start(
                        topall[:, :].rearrange("b (c v) -> b c v", v=4),
                        cc_top_out[ts].rearrange("(c b) v -> b c v", b=B),
                    )
                    t3 = topall[:, :].rearrange("b (c v) -> b c v", v=4)
                    if ts + 1 < n_dec:
                        gv = wk.tile([B, 1], F32, tag="gv", bufs=2, name=f"gv{ts}")
                        nc.vector.tensor_reduce(gv[:, :], t3[:, :, 0],
                                                axis=mybir.AxisListType.X, op=ALU.max)
                        wtokf = wk.tile([B, NCORE], F32, tag="wtokf", bufs=2,
                                        name=f"wtokf{ts}")
                        nc.vector.scalar_tensor_tensor(wtokf[:, :], t3[:, :, 0], gv[:, :],
                                                       t3[:, :, 1], ALU.is_equal, ALU.mult)
                        wtok = wk.tile([B, 1], F32, tag="wtok", bufs=2, name=f"wtok{ts}")
                        nc.vector.tensor_reduce(wtok[:, :], wtokf[:, :],
                                                axis=mybir.AxisListType.X, op=ALU.max)
                        wtoki = wk.tile([B, 1], DT.int32, tag="wtoki", bufs=2,
                                        name=f"wtoki{ts}")
                        nc.vector.tensor_copy(wtoki[:, :], wtok[:, :])
                    # total expsum over cores -> -lse via fast-log + one Newton
                    sxt = wk.tile([B, 1], F32, tag="sxt", bufs=2, name=f"sxt{ts}")
                    nc.vector.tensor_reduce(sxt[:, :], t3[:, :, 2], axis=mybir.AxisListType.X,
                                            op=ALU.add)
                    si = wk.tile([B, 1], F32, tag="si", bufs=2, name=f"si{ts}")
                    nc.vector.tensor_copy(si[:, :], sxt[:, :].bitcast(DT.int32))
                    y0 = wk.tile([B, 1], F32, tag="y0", bufs=2, name=f"y0{ts}")
                    nc.vector.tensor_scalar(y0[:, :], si[:, :], LOGB, LOGK,
                                            ALU.subtract, ALU.mult)
                    ee = wk.tile([B, 1], F32, tag="ee", bufs=2, name=f"ee{ts}")
                    nc.scalar.activation(ee[:, :], y0[:, :], AF.Exp, scale=-1.0)
                    zz = wk.tile([B, 1], F32, tag="zz", bufs=2, name=f"zz{ts}")
                    nc.vector.tensor_mul(zz[:, :], sxt[:, :], ee[:, :])
                    nlzneg = wk.tile([B, 1], F32, tag="nlzneg", bufs=2, name=f"nlz{ts}")
                    # -lse = -(y0 + (z - 1)) = (1 - z) - y0
                    nc.vector.scalar_tensor_tensor(nlzneg[:, :], zz[:, :], 1.0, y0[:, :],
                                                   ALU.subtract, ALU.add)
                    nc.vector.tensor_scalar_mul(nlzneg[:, :], nlzneg[:, :], -1.0)

                    # --- next-token embedding gather + transpose ---
                    if ts + 1 < n_dec:
                        lemb = wk.tile([B, E], BF, tag="lemb", bufs=2, name=f"lemb{ts}")
                        nc.gpsimd.indirect_dma_start(
                            out=lemb[:, :], out_offset=None, in_=io["embed"][:, :],
                            in_offset=bass.IndirectOffsetOnAxis(ap=wtoki[:, :1], axis=0),
                        )
                        newl = wk.tile([128, 2 * B], BF, tag="lastT", bufs=2,
                                       name=f"lastT{ts}")
                        for h in range(2):
                            pt = ps.tile([128, B], BF, tag="bank",
                                         name=f"lt{ts}_{h}")
                            nc.tensor.transpose(pt[:, :], lemb[:, h * 128:(h + 1) * 128],
                                                identB[:B, :B])
                            evict(newl[:, h * B:(h + 1) * B], pt[:, :])
                        lastT = newl

                    # overlapped normalize+write of the previous step
                    if prev_store is not None:
                        write_step(ts - 1, prev_store, prev_nlz)
                    prev_store = store[:, :]
                    prev_nlz = nlzneg[:, :]

                if prev_store is not None:
                    write_step(n_dec - 1, prev_store, prev_nlz)

    nc.finalize()
    return nc


def prep_inputs(inputs):
    """Host-side shard/pack. Returns in_maps list for the 8 cores."""
    f32 = np.float32
    bfnp = DT.np(BF)
    emb = np.asarray(inputs["embed_w"], dtype=f32).astype(bfnp)
    packs = {}
    for g in GK:
        wih = np.asarray(inputs[f"{g}_wih"], dtype=f32)
        whh = np.asarray(inputs[f"{g}_whh"], dtype=f32)
        bih = np.asarray(inputs[f"{g}_bih"], dtype=f32)
        bhh = np.asarray(inputs[f"{g}_bhh"], dtype=f32)
        packs[f"{g}_wihT"] = np.ascontiguousarray(wih.T).astype(bfnp)
        packs[f"{g}_whhT"] = np.ascontiguousarray(whh.T).astype(bfnp)
        brz6 = np.empty((128, 6), f32)
        for m in range(4):
            brz6[:, m] = bih[m * 128:(m + 1) * 128] + bhh[m * 128:(m + 1) * 128]
        for hh in range(2):
            brz6[:, 4 + hh] = bih[512 + hh * 128:512 + (hh + 1) * 128]
        packs[f"{g}_brz6"] = brz6
        packs[f"{g}_bhn"] = np.ascontiguousarray(bhh[512:768].reshape(2, 128).T)
    packs["g1T"] = np.ascontiguousarray(np.asarray(inputs["gate_w1"], f32).T).astype(bfnp)
    packs["g2T"] = np.ascontiguousarray(np.asarray(inputs["gate_w2"], f32).T).astype(bfnp)
    packs["gb1"] = np.ascontiguousarray(np.asarray(inputs["gate_b1"], f32).reshape(2, 128).T)
    packs["gb2"] = np.asarray(inputs["gate_b2"], f32).reshape(1, 1)
    # memory-GRU constant bias tiles (x side has no hoisted projection)
    memb = packs["mem_brz6"]
    packs["memrz"] = np.ascontiguousarray(np.repeat(memb[:, 0:4], BB, axis=1), f32)
    packs["membin"] = np.ascontiguousarray(np.repeat(memb[:, 4:6], BB, axis=1), f32)
    fcwT = np.ascontiguousarray(np.asarray(inputs["fc_w"], f32).T)
    fcb = np.asarray(inputs["fc_b"], f32)
    fcb_nonzero = bool(np.any(fcb != 0))
    e1 = np.asarray(inputs["embed_w"], f32)[SEQBEGIN].astype(bfnp)
    l0 = np.empty((128, 2 * B), bfnp)
    for k in range(2):
        l0[:, k * B:(k + 1) * B] = np.tile(e1[k * 128:(k + 1) * 128][:, None], (1, B))
    allfacts = np.asarray(inputs["allfacts"], np.int32)
    questions = np.asarray(inputs["questions"], np.int32)

    in_maps = []
    for j in range(NCORE):
        m = dict(packs)
        m["embed"] = emb
        m["fcwT"] = np.ascontiguousarray(fcwT[:, j * VS:(j + 1) * VS]).astype(bfnp)
        if fcb_nonzero:
            m["fcb"] = np.ascontiguousarray(np.tile(fcb[None, j * VS:(j + 1) * VS], (B, 1)))
        m["l0"] = l0
        m["voff"] = np.full((B, 1), j * VS, f32)
        m["facts_idx"] = np.ascontiguousarray(
            allfacts[j * BB:(j + 1) * BB].reshape(NSEQ, FL).T.reshape(-1, 1))
        m["q_idx"] = np.ascontiguousarray(
            questions[j * BB:(j + 1) * BB].reshape(BB, QL).T.reshape(-1, 1))
        in_maps.append(m)
    return in_maps, fcb_nonzero


_CACHE = {}


def kernel(**inputs):
    alen = int(inputs["alen"])
    in_maps, fcb_nonzero = prep_inputs(inputs)
    key = (alen, fcb_nonzero)
    if key not in _CACHE:
        _CACHE[key] = build_nc(alen, fcb_nonzero)
    nc = _CACHE[key]
    res = run_bass_kernel_spmd(nc, in_maps, core_ids=list(range(NCORE)))
    out = np.concatenate([res.results[j]["out_logp"] for j in range(NCORE)], axis=1)
    return out.astype(np.float32)


# revision 8
# speedup vs baseline: 2.0839x; 1.0831x over previous
"""DMN forward on 8 Trainium2 NeuronCores (Bass/Tile), bf16 matmul build.

Sharding: batch rows 8/core for fact+question encoding and episodic memory
(core j owns batch rows 8j..8j+7 and their 160 fact sequences); decode GRU
replicated on all cores, fc/log-softmax vocab-sharded 4000 columns/core, with
one small AllGather per decode step carrying (max, argmax-token, expsum) for
the greedy feedback and the log-softmax normalizer.

All matmuls run in bf16 (PE single-pass) with fp32 PSUM accumulation; GRU
non-linearities use the native Sigmoid table in the encoder and the tanh form
in decode (so Exp shares the same table).  Each GRU step packs its six gate
blocks into two PSUM banks (r/z and n) as column groups, so the elementwise
tail is ~9 fused ops.  The fact-token projection is hoisted, SBUF-resident
(bf16), biases folded in at PSUM-evict time, and its gathers/transposes/
matmuls are emitted software-pipelined with the fact GRU steps.  Decode is
staggered: the recurrent-path matmuls of step t+1, the exp/normalizer work
(lagged one step), and the previous step's output writes all execute inside
step t's AllGather window.  The log-softmax normalizer uses a bitcast
fast-log plus one exp-Newton step instead of Ln, keeping the scalar engine
on a single activation table through decode.
"""

import os
import numpy as np

import concourse.bass as bass
import concourse.bacc as bacc
import concourse.mybir as mybir
from concourse.tile import TileContext
from concourse.bass_utils import run_bass_kernel_spmd
from concourse.masks import make_identity

AF = mybir.ActivationFunctionType
ALU = mybir.AluOpType
DT = mybir.dt
BF = DT.bfloat16
F32 = DT.float32

V, E, H = 32000, 256, 256
B, NF, FL, QL = 64, 20, 32, 16
N_EPISODE = 3
SEQBEGIN = 1
NCORE = 8
BB = B // NCORE            # batch rows per core = 8
NSEQ = BB * NF             # fact seqs per core = 160
NTOK = NSEQ * FL           # fact tokens per core = 5120
VS = V // NCORE            # vocab shard = 4000
NCHUNK = 8
CHW = VS // NCHUNK         # 500
FH = NSEQ // 2             # fact chain width = 80
VH = VS // 2               # argmax half-scan width = 2000

GK = {"ig": E, "qg": E, "att": H, "mem": H, "ans": 2 * H}

# fast-log constants: ln(x) ~ (bitcast_i32(x) - LOGB) * LOGK, refined by one
# Newton step y <- y + (x*exp(-y) - 1)
LOGK = float(np.log(2.0) / (1 << 23))
LOGB = 1064866805.0


def build_nc(alen, fcb_nonzero):
    nc = bacc.Bacc("TRN2", num_devices=NCORE)

    def dram_in(name, shape, dtype=F32):
        return nc.dram_tensor(name, list(shape), dtype, kind="ExternalInput")

    io = {}
    io["facts_idx"] = dram_in("facts_idx", [NTOK, 1], DT.int32)
    io["q_idx"] = dram_in("q_idx", [BB * QL, 1], DT.int32)
    io["embed"] = dram_in("embed", [V, E], BF)
    io["fcwT"] = dram_in("fcwT", [E, VS], BF)
    io["l0"] = dram_in("l0", [128, 2 * B], BF)
    io["voff"] = dram_in("voff", [B, 1])
    if fcb_nonzero:
        io["fcb"] = dram_in("fcb", [B, VS])
    for g, kin in GK.items():
        io[f"{g}_wihT"] = dram_in(f"{g}_wihT", [kin, 3 * H], BF)
        io[f"{g}_whhT"] = dram_in(f"{g}_whhT", [H, 3 * H], BF)
        io[f"{g}_brz6"] = dram_in(f"{g}_brz6", [128, 6])
        io[f"{g}_bhn"] = dram_in(f"{g}_bhn", [128, 2])
    io["g1T"] = dram_in("g1T", [4 * H, H], BF)
    io["g2T"] = dram_in("g2T", [H, 1], BF)
    io["gb1"] = dram_in("gb1", [128, 2])
    io["gb2"] = dram_in("gb2", [1, 1])
    io["memrz"] = dram_in("memrz", [128, 4 * BB])
    io["membin"] = dram_in("membin", [128, 2 * BB])

    out_logp = nc.dram_tensor("out_logp", [B * alen, VS], F32, kind="ExternalOutput")

    cc_enc_in = nc.dram_tensor("cc_enc_in", [BB, 2 * H], F32, kind="Internal")
    cc_enc_out = nc.dram_tensor("cc_enc_out", [B, 2 * H], F32, kind="Internal", addr_space="Shared")
    n_cc = alen + 1
    cc_top_in = [nc.dram_tensor(f"cc_top_in{t}", [B, 4], F32, kind="Internal") for t in range(n_cc)]
    cc_top_out = [nc.dram_tensor(f"cc_top_out{t}", [NCORE * B, 4], F32, kind="Internal",
                                 addr_space="Shared") for t in range(n_cc)]
    rg = [list(range(NCORE))]

    dbg = int(os.environ.get("K_DEBUG_STEPS", "0"))
    n_fl = dbg or FL
    n_ql = dbg or QL
    n_nf = dbg or NF
    n_ep = 1 if dbg else N_EPISODE
    n_dec = min(alen, dbg) if dbg else alen

    with TileContext(nc) as tc:
        with tc.tile_pool(name="shared", bufs=1) as shp, \
             tc.tile_pool(name="state", bufs=1) as st, \
             tc.tile_pool(name="work", bufs=3) as wk, \
             tc.tile_pool(name="ps", bufs=8, space="PSUM") as ps:

            identB = shp.tile([128, 128], BF)
            make_identity(nc, identB[:, :])
            identF = shp.tile([128, 128], F32)
            make_identity(nc, identF[:, :])

            W = {}

            def load_gate(pool, g):
                kin = GK[g]
                xw = []
                for k in range(kin // 128):
                    t = pool.tile([128, 3 * H], BF, name=f"{g}xw{k}")
                    nc.sync.dma_start(t[:, :], io[f"{g}_wihT"][k * 128:(k + 1) * 128, :])
                    xw.append(t)
                hw = []
                for k in range(2):
                    t = pool.tile([128, 3 * H], BF, name=f"{g}hw{k}")
                    nc.sync.dma_start(t[:, :], io[f"{g}_whhT"][k * 128:(k + 1) * 128, :])
                    hw.append(t)
                brz = pool.tile([128, 6], F32, name=f"{g}brz")
                nc.sync.dma_start(brz[:, :], io[f"{g}_brz6"][:, :])
                bhn = pool.tile([128, 2], F32, name=f"{g}bhn")
                nc.sync.dma_start(bhn[:, :], io[f"{g}_bhn"][:, :])
                W[g] = (xw, hw, brz, bhn)

            evict_rr = [0]

            def evict(dst_ap, src_ap, bias=None):
                if bias is None:
                    if evict_rr[0] % 2 == 0:
                        nc.vector.tensor_copy(dst_ap, src_ap)
                    else:
                        nc.scalar.activation(dst_ap, src_ap, AF.Copy)
                else:
                    if evict_rr[0] % 2 == 0:
                        nc.vector.tensor_scalar(dst_ap, src_ap, bias, None, ALU.add)
                    else:
                        nc.scalar.activation(dst_ap, src_ap, AF.Identity, bias=bias)
                evict_rr[0] += 1

            # ---- one fused GRU step -------------------------------------
            # brz: psum [128, 4F] (r0 r1 z0 z1), bnh: psum [128, 2F] (n0 n1)
            # h: [128, 2F] bf16 (hidden half k at cols kF).
            def gru_mms(brz, bnh, g, h_ap, F, x_rhs=None, xw_override=None):
                xw, hw, _, _ = W[g]
                if xw_override is not None:
                    xw = xw_override

                def dst(m):
                    if m < 4:
                        return brz[:, m * F:(m + 1) * F]
                    return bnh[:, (m - 4) * F:(m - 3) * F]

                for m in range(6):
                    first = True
                    if x_rhs is not None:
                        for k in range(len(x_rhs)):
                            nc.tensor.matmul(dst(m), xw[k][:, m * 128:(m + 1) * 128],
                                             x_rhs[k], start=first, stop=False)
                            first = False
                    for k in range(2):
                        nc.tensor.matmul(dst(m), hw[k][:, m * 128:(m + 1) * 128],
                                         h_ap[:, k * F:(k + 1) * F],
                                         start=first, stop=(k == 1))
                        first = False

            def gru_ew(brz, bnh, g, h_ap, F, gi_rz, gi_n, name, sig_direct=True,
                       att=None):
                """gi_rz: AP [128, 4F]-size (x-proj + all r/z biases);
                gi_n: AP [128, 2F]-size (x-proj n + bih_n).
                att: None for plain GRU; (e_ap, negg_ap) for episode form."""
                _, _, _, bhn = W[g]
                trz = wk.tile([128, 4 * F], BF, tag=f"trz{F}{name[0]}", bufs=2,
                              name=f"{name}trz")
                nc.vector.tensor_add(trz[:, :].rearrange("p (m f) -> p m f", f=F),
                                     brz[:, 0:4 * F].rearrange("p (m f) -> p m f", f=F),
                                     gi_rz)
                rz = wk.tile([128, 4 * F], BF, tag=f"rz{F}{name[0]}", bufs=2,
                             name=f"{name}rz")
                if sig_direct:
                    nc.scalar.activation(rz[:, :], trz[:, :], AF.Sigmoid)
                else:
                    th = wk.tile([128, 4 * F], BF, tag=f"th{F}{name[0]}", bufs=2,
                                 name=f"{name}th")
                    nc.scalar.activation(th[:, :], trz[:, :], AF.Tanh, scale=0.5)
                    nc.vector.tensor_scalar(rz[:, :], th[:, :], 0.5, 0.5,
                                            ALU.mult, ALU.add)
                y = wk.tile([128, 2 * F], BF, tag=f"y{F}{name[0]}", bufs=2,
                            name=f"{name}y")
                for h in range(2):
                    nc.vector.scalar_tensor_tensor(
                        y[:, h * F:(h + 1) * F], bnh[:, h * F:(h + 1) * F],
                        bhn[:, h:h + 1], rz[:, h * F:(h + 1) * F], ALU.add, ALU.mult)
                u = wk.tile([128, 2 * F], BF, tag=f"u{F}{name[0]}", bufs=2,
                            name=f"{name}u")
                nc.vector.tensor_add(u[:, :].rearrange("p (m f) -> p m f", f=F),
                                     y[:, :].rearrange("p (m f) -> p m f", f=F), gi_n)
                n = wk.tile([128, 2 * F], BF, tag=f"n{F}{name[0]}", bufs=2,
                            name=f"{name}n")
                nc.scalar.activation(n[:, :], u[:, :], AF.Tanh)
                if att is None:
                    # h' = n + z * (h - n)
                    d = wk.tile([128, 2 * F], BF, tag=f"d{F}{name[0]}", bufs=2,
                                name=f"{name}d")
                    nc.vector.tensor_sub(d[:, :], h_ap, n[:, :])
                    w2 = wk.tile([128, 2 * F], BF, tag=f"w{F}{name[0]}", bufs=2,
                                 name=f"{name}w")
                    nc.vector.tensor_mul(w2[:, :], rz[:, 2 * F:4 * F], d[:, :])
                    hn = wk.tile([128, 2 * F], BF, tag=f"hn{F}{name[0]}", bufs=2,
                                 name=f"{name}hn")
                    nc.vector.tensor_add(hn[:, :], n[:, :], w2[:, :])
                    return hn
                else:
                    # e' = e + g*(1-z)*(n - e);  w1 = (z - 1)*(-g)
                    e_ap, negg = att
                    d = wk.tile([128, 2 * F], BF, tag=f"d{F}{name[0]}", bufs=2,
                                name=f"{name}d")
                    nc.vector.tensor_sub(d[:, :], n[:, :], e_ap)
                    w1 = wk.tile([128, 2 * F], BF, tag=f"w1{F}{name[0]}", bufs=2,
                                 name=f"{name}w1")
                    nc.vector.scalar_tensor_tensor(
                        w1[:, :].rearrange("p (m f) -> p m f", f=F),
                        rz[:, 2 * F:4 * F].rearrange("p (m f) -> p m f", f=F),
                        1.0, negg, ALU.subtract, ALU.mult)
                    p2 = wk.tile([128, 2 * F], BF, tag=f"p2{F}{name[0]}", bufs=2,
                                 name=f"{name}p2")
                    nc.vector.tensor_mul(p2[:, :], w1[:, :], d[:, :])
                    en = wk.tile([128, 2 * F], BF, tag=f"hn{F}{name[0]}", bufs=2,
                                 name=f"{name}en")
                    nc.vector.tensor_add(en[:, :], e_ap, p2[:, :])
                    return en

            # ========== P1+P2+P3: pipelined gather/x-proj + GRUs =========
            with tc.tile_pool(name="fpool", bufs=1) as fp:
                load_gate(fp, "ig")
                load_gate(fp, "qg")
                XT = [fp.tile([128, NTOK], BF, name=f"XT{k}") for k in range(2)]
                fidx = fp.tile([128, NTOK // 128], DT.int32, name="fidx")
                nc.sync.dma_start(fidx[:, :], io["facts_idx"].rearrange("(b a) o -> a (b o)", a=128))
                qidx = wk.tile([128, 1], DT.int32, name="qidx")
                nc.sync.dma_start(qidx[:, :], io["q_idx"][:, :])

                gi = fp.tile([128, 6 * NTOK], BF, name="gi")
                gi3 = gi[:, :].rearrange("p (m t) -> p m t", t=NTOK)
                ig_brz = W["ig"][2]

                # question gather + x-projection first (tiny, unblocks q GRU)
                qg_t = wk.tile([128, E], BF, tag="fgat", bufs=4, name="qgat")
                nc.gpsimd.indirect_dma_start(
                    out=qg_t[:, :], out_offset=None, in_=io["embed"][:, :],
                    in_offset=bass.IndirectOffsetOnAxis(ap=qidx[:, :1], axis=0),
                )
                XQ = fp.tile([128, 2 * BB * QL], BF, name="XQ")
                for ch in range(2):
                    pt = ps.tile([128, 128], BF, tag="bank", name=f"qtp{ch}")
                    nc.tensor.transpose(pt[:, :], qg_t[:, ch * 128:(ch + 1) * 128], identB[:, :])
                    evict(XQ[:, ch * 128:(ch + 1) * 128], pt[:, :])
                giq = fp.tile([128, 6 * BB * QL], BF, name="giq")
                giq3 = giq[:, :].rearrange("p (m t) -> p m t", t=BB * QL)
                qg_brz = W["qg"][2]
                for m in range(6):
                    pp = ps.tile([128, BB * QL], F32, tag="bank", name=f"qxp{m}")
                    for k in range(2):
                        nc.tensor.matmul(pp[:, :], W["qg"][0][k][:, m * 128:(m + 1) * 128],
                                         XQ[:, k * 128:(k + 1) * 128], start=(k == 0), stop=(k == 1))
                    evict(giq3[:, m, :], pp[:, :], bias=qg_brz[:, m:m + 1])

                NG = NTOK // 128     # 40 gathers
                NCH = NTOK // 512    # 10 x-proj chunks

                def emit_gather(i):
                    gt = wk.tile([128, E], BF, tag="fgat", bufs=4, name=f"fg{i}")
                    nc.gpsimd.indirect_dma_start(
                        out=gt[:, :], out_offset=None, in_=io["embed"][:, :],
                        in_offset=bass.IndirectOffsetOnAxis(ap=fidx[:, i:i + 1], axis=0),
                    )
                    for ch in range(2):
                        pt = ps.tile([128, 128], BF, tag="bank", name=f"ftp{i}_{ch}")
                        nc.tensor.transpose(pt[:, :], gt[:, ch * 128:(ch + 1) * 128], identB[:, :])
                        evict(XT[ch][:, i * 128:(i + 1) * 128], pt[:, :])

                def emit_xchunk(c):
                    for m in range(6):
                        pp = ps.tile([128, 512], F32, tag="bank", name=f"xp{m}_{c}")
                        for k in range(2):
                            nc.tensor.matmul(pp[:, :], W["ig"][0][k][:, m * 128:(m + 1) * 128],
                                             XT[k][:, c * 512:(c + 1) * 512],
                                             start=(k == 0), stop=(k == 1))
                        evict(gi3[:, m, c * 512:(c + 1) * 512], pp[:, :],
                              bias=ig_brz[:, m:m + 1])

                g_done = 0
                c_done = 0

                hA = wk.tile([128, NSEQ], BF, tag="hA", bufs=2, name="hA0")
                hB = wk.tile([128, NSEQ], BF, tag="hB", bufs=2, name="hB0")
                hq = wk.tile([128, 2 * BB], BF, tag="hq", bufs=2, name="hq0")
                nc.vector.memset(hA[:, :], 0.0)
                nc.vector.memset(hB[:, :], 0.0)
                nc.vector.memset(hq[:, :], 0.0)
                for t in range(n_fl):
                    # stay ~3 GRU steps ahead with gathers / x-proj chunks
                    need_tok = min(NTOK, (t + 3) * NSEQ)
                    while g_done < NG and g_done * 128 < min(NTOK, need_tok + 512):
                        emit_gather(g_done)
                        g_done += 1
                    while c_done < NCH and c_done * 512 < need_tok:
                        emit_xchunk(c_done)
                        c_done += 1
                    bArz = ps.tile([128, 4 * FH], F32, tag="bank", name=f"bArz{t}")
                    bAnh = ps.tile([128, 2 * FH], F32, tag="bank", name=f"bAnh{t}")
                    gru_mms(bArz, bAnh, "ig", hA[:, :], FH)
                    bBrz = ps.tile([128, 4 * FH], F32, tag="bank", name=f"bBrz{t}")
                    bBnh = ps.tile([128, 2 * FH], F32, tag="bank", name=f"bBnh{t}")
                    gru_mms(bBrz, bBnh, "ig", hB[:, :], FH)
                    do_q = (t % 2 == 0) and (t // 2 < n_ql)
                    if do_q:
                        tq = t // 2
                        bQrz = ps.tile([128, 4 * BB], F32, tag="bank", name=f"bQrz{tq}")
                        bQnh = ps.tile([128, 2 * BB], F32, tag="bank", name=f"bQnh{tq}")
                        gru_mms(bQrz, bQnh, "qg", hq[:, :], BB)
                    t0 = t * NSEQ
                    hA = gru_ew(bArz, bAnh, "ig", hA[:, :], FH,
                                gi3[:, 0:4, t0:t0 + FH], gi3[:, 4:6, t0:t0 + FH],
                                name=f"A{t}_")
                    hB = gru_ew(bBrz, bBnh, "ig", hB[:, :], FH,
                                gi3[:, 0:4, t0 + FH:t0 + NSEQ],
                                gi3[:, 4:6, t0 + FH:t0 + NSEQ], name=f"B{t}_")
                    if do_q:
                        q0 = tq * BB
                        hq = gru_ew(bQrz, bQnh, "qg", hq[:, :], BB,
                                    giq3[:, 0:4, q0:q0 + BB], giq3[:, 4:6, q0:q0 + BB],
                                    name=f"Q{tq}_")

                # persist enc_facts [128, 2, NSEQ] and enc_q [128, 2*BB]
                encf = st.tile([128, 2 * NSEQ], BF, name="encf")
                for h in range(2):
                    nc.vector.tensor_copy(encf[:, h * NSEQ:h * NSEQ + FH],
                                          hA[:, h * FH:(h + 1) * FH])
                    nc.vector.tensor_copy(encf[:, h * NSEQ + FH:(h + 1) * NSEQ],
                                          hB[:, h * FH:(h + 1) * FH])
                hqF = st.tile([128, 2 * BB], BF, name="hqF")
                nc.vector.tensor_copy(hqF[:, :], hq[:, :])

            # ================= P4: episodes =================
            with tc.tile_pool(name="epool", bufs=1) as epl:
                load_gate(epl, "att")
                load_gate(epl, "mem")
                g1T = []
                for k in range(8):
                    tt = epl.tile([128, H], BF, name=f"g1T{k}")
                    nc.sync.dma_start(tt[:, :], io["g1T"][k * 128:(k + 1) * 128, :])
                    g1T.append(tt)
                g2T = []
                for k in range(2):
                    tt = epl.tile([128, 1], BF, name=f"g2T{k}")
                    nc.sync.dma_start(tt[:, :], io["g2T"][k * 128:(k + 1) * 128, :])
                    g2T.append(tt)
                gb1 = epl.tile([128, 2], F32)
                nc.sync.dma_start(gb1[:, :], io["gb1"][:, :])
                gb2 = epl.tile([1, 1], F32)
                nc.sync.dma_start(gb2[:, :], io["gb2"][:, :])
                memrz = epl.tile([128, 4 * BB], F32)
                nc.sync.dma_start(memrz[:, :], io["memrz"][:, :])
                membin = epl.tile([128, 2 * BB], F32)
                nc.sync.dma_start(membin[:, :], io["membin"][:, :])

                # att x-projection of enc_facts (+ biases)
                giaP = epl.tile([128, 6 * NSEQ], BF, name="giaP")
                gia3 = giaP[:, :].rearrange("p (m t) -> p m t", t=NSEQ)
                att_brz = W["att"][2]
                for m in range(6):
                    pp = ps.tile([128, NSEQ], F32, tag="bank", name=f"axp{m}")
                    for k in range(2):
                        nc.tensor.matmul(pp[:, :], W["att"][0][k][:, m * 128:(m + 1) * 128],
                                         encf[:, k * NSEQ:(k + 1) * NSEQ],
                                         start=(k == 0), stop=(k == 1))
                    evict(gia3[:, m, :], pp[:, :], bias=att_brz[:, m:m + 1])
                gia4 = giaP[:, :].rearrange("p (m b i) -> p m b i", m=6, i=NF)

                memT = wk.tile([128, 2 * BB], BF, tag="memT", bufs=2, name="memT0")
                nc.vector.tensor_copy(memT[:, :], hqF[:, :])
                encf3 = [encf[:, k * NSEQ:(k + 1) * NSEQ].rearrange("p (r i) -> p r i", i=NF)
                         for k in range(2)]

                for ep in range(n_ep):
                    # gate features z = [f*q, f*m, |f-q|, |f-m|]
                    ZT = [wk.tile([128, NSEQ], BF, tag=f"zt{x}", bufs=1, name=f"ZT{ep}_{x}")
                          for x in range(8)]
                    for h in range(2):
                        qb = hqF[:, h * BB:(h + 1) * BB].to_broadcast([128, BB, NF])
                        mb = memT[:, h * BB:(h + 1) * BB].to_broadcast([128, BB, NF])
                        z3 = [ZT[x][:, :].rearrange("p (r i) -> p r i", i=NF) for x in range(8)]
                        nc.vector.tensor_mul(z3[0 + h], encf3[h], qb)
                        nc.vector.tensor_mul(z3[2 + h], encf3[h], mb)
                        dq = wk.tile([128, NSEQ], F32, tag="dq", bufs=2, name=f"dq{ep}_{h}")
                        nc.vector.tensor_sub(dq[:, :].rearrange("p (r i) -> p r i", i=NF),
                                             encf3[h], qb)
                        nc.scalar.activation(ZT[4 + h][:, :], dq[:, :], AF.Abs)
                        dm = wk.tile([128, NSEQ], F32, tag="dm", bufs=2, name=f"dm{ep}_{h}")
                        nc.vector.tensor_sub(dm[:, :].rearrange("p (r i) -> p r i", i=NF),
                                             encf3[h], mb)
                        nc.scalar.activation(ZT[6 + h][:, :], dm[:, :], AF.Abs)
                    p1T = []
                    for m in range(2):
                        pp = ps.tile([128, NSEQ], F32, tag="bank", name=f"p1{ep}_{m}")
                        for k in range(8):
                            nc.tensor.matmul(pp[:, :], g1T[k][:, m * 128:(m + 1) * 128],
                                             ZT[k][:, :], start=(k == 0), stop=(k == 7))
                        t1 = wk.tile([128, NSEQ], BF, tag="p1s", bufs=2, name=f"p1s{ep}_{m}")
                        nc.scalar.activation(t1[:, :], pp[:, :], AF.Tanh, bias=gb1[:, m:m + 1])
                        p1T.append(t1)
                    pgp = ps.tile([1, NSEQ], F32, tag="bank", name=f"pg{ep}")
                    for k in range(2):
                        nc.tensor.matmul(pgp[:, :], g2T[k][:, :], p1T[k][:, :],
                                         start=(k == 0), stop=(k == 1))
                    g_row = wk.tile([1, NSEQ], F32, tag="grow", bufs=1, name=f"grow{ep}")
                    nc.scalar.activation(g_row[:, :], pgp[:, :], AF.Sigmoid, bias=gb2[:1, :1])
                    ngrow = wk.tile([1, NSEQ], F32, tag="ngrow", bufs=1, name=f"ngrow{ep}")
                    nc.vector.tensor_scalar_mul(ngrow[:, :], g_row[:, :], -1.0)
                    negG = wk.tile([128, 2 * NSEQ], F32, tag="negG", bufs=1, name=f"negG{ep}")
                    nc.gpsimd.partition_broadcast(negG[:, 0:NSEQ], ngrow[:, :])
                    nc.vector.tensor_copy(negG[:, NSEQ:2 * NSEQ], negG[:, 0:NSEQ])
                    negG4 = negG[:, :].rearrange("p (h b i) -> p h b i", h=2, i=NF)

                    eT = wk.tile([128, 2 * BB], BF, tag="eT", bufs=2, name=f"eT{ep}")
                    nc.vector.memset(eT[:, :], 0.0)
                    for i in range(n_nf):
                        erz = ps.tile([128, 4 * BB], F32, tag="bank", name=f"erz{ep}_{i}")
                        enh = ps.tile([128, 2 * BB], F32, tag="bank", name=f"enh{ep}_{i}")
                        gru_mms(erz, enh, "att", eT[:, :], BB)
                        eT = gru_ew(erz, enh, "att", eT[:, :], BB,
                                    gia4[:, 0:4, :, i], gia4[:, 4:6, :, i],
                                    name=f"e{ep}_{i}_", att=(eT[:, :], negG4[:, :, :, i]))
                    # memory GRU step (x = eT)
                    mrz = ps.tile([128, 4 * BB], F32, tag="bank", name=f"mrz{ep}")
                    mnh = ps.tile([128, 2 * BB], F32, tag="bank", name=f"mnh{ep}")
                    gru_mms(mrz, mnh, "mem", memT[:, :], BB,
                            x_rhs=[eT[:, k * BB:(k + 1) * BB] for k in range(2)])
                    memT = gru_ew(mrz, mnh, "mem", memT[:, :], BB,
                                  memrz[:, :].rearrange("p (m f) -> p m f", f=BB),
                                  membin[:, :].rearrange("p (m f) -> p m f", f=BB),
                                  name=f"m{ep}_")

                memF = st.tile([128, 2 * BB], BF, name="memF")
                nc.vector.tensor_copy(memF[:, :], memT[:, :])

            # ================= P5: all-gather mem|enc_q =================
            encrow = wk.tile([BB, 2 * H], F32, name="encrow")
            for h in range(2):
                pt = ps.tile([BB, 128], BF, tag="bank", name=f"egm{h}")
                nc.tensor.transpose(pt[:, :], memF[:, h * BB:(h + 1) * BB], identB[:, :])
                evict(encrow[:, h * 128:(h + 1) * 128], pt[:, :])
                pt2 = ps.tile([BB, 128], BF, tag="bank", name=f"egq{h}")
                nc.tensor.transpose(pt2[:, :], hqF[:, h * BB:(h + 1) * BB], identB[:, :])
                evict(encrow[:, 256 + h * 128:256 + (h + 1) * 128], pt2[:, :])
            nc.sync.dma_start(cc_enc_in[:, :], encrow[:, :])
            nc.gpsimd.collective_compute("AllGather", ALU.bypass, ins=[cc_enc_in[:, :]],
                                         outs=[cc_enc_out[:, :]], replica_groups=rg)
            enc_all = wk.tile([B, 2 * H], F32, name="enc_all")
            nc.sync.dma_start(enc_all[:, :], cc_enc_out[:, :])

            # ================= P6: decode =================
            with tc.tile_pool(name="dpool", bufs=1) as dp:
                load_gate(dp, "ans")
                fcwT = []
                for k in range(2):
                    tt = dp.tile([128, VS], BF, name=f"fcwT{k}")
                    nc.sync.dma_start(tt[:, :], io["fcwT"][k * 128:(k + 1) * 128, :])
                    fcwT.append(tt)
                l0 = dp.tile([128, 2 * B], BF, name="l0")
                nc.sync.dma_start(l0[:, :], io["l0"][:, :])
                vofft = dp.tile([B, 1], F32)
                nc.sync.dma_start(vofft[:, :], io["voff"][:, :])
                iota_i = dp.tile([B, VS], DT.int32)
                nc.gpsimd.iota(iota_i[:, :], pattern=[[1, VS]], base=0, channel_multiplier=0)
                iotaG = dp.tile([B, VS], F32)
                nc.vector.tensor_copy(iotaG[:, :], iota_i[:, :])

                # transposed mem/enc_q for all 64 rows
                hid = wk.tile([128, 2 * B], BF, tag="hid", bufs=2, name="hid0")
                qA = dp.tile([128, 2 * B], BF, name="qA")
                for h in range(2):
                    pt = ps.tile([128, B], F32, tag="bank", name=f"tmA{h}")
                    nc.tensor.transpose(pt[:, :], enc_all[:, h * 128:(h + 1) * 128],
                                        identF[:B, :B])
                    evict(hid[:, h * B:(h + 1) * B], pt[:, :])
                    pt2 = ps.tile([128, B], F32, tag="bank", name=f"tqA{h}")
                    nc.tensor.transpose(pt2[:, :], enc_all[:, 256 + h * 128:256 + (h + 1) * 128],
                                        identF[:B, :B])
                    evict(qA[:, h * B:(h + 1) * B], pt2[:, :])

                # enc_q half of the ans input projection (+ all r/z/n biases)
                giq6 = dp.tile([128, 6 * B], BF, name="giq6")
                giq63 = giq6[:, :].rearrange("p (m t) -> p m t", t=B)
                ans_xw = W["ans"][0]
                ans_brz = W["ans"][2]
                for m in range(6):
                    pp = ps.tile([128, B], F32, tag="bank", name=f"dxp{m}")
                    for k in range(2):
                        nc.tensor.matmul(pp[:, :], ans_xw[2 + k][:, m * 128:(m + 1) * 128],
                                         qA[:, k * B:(k + 1) * B], start=(k == 0), stop=(k == 1))
                    evict(giq63[:, m, :], pp[:, :], bias=ans_brz[:, m:m + 1])

                lastT = l0
                out3 = out_logp.rearrange("(b t) v -> b t v", t=alen)
                stores = {}
                sxs_prev = None

                def write_step(wts, nlzneg):
                    sv = stores.pop(wts)
                    for vv in range(2):
                        sl = sv[:, vv * VH:(vv + 1) * VH]
                        ot = dp.tile([B, VH], F32, tag=f"ot{vv}", bufs=2,
                                     name=f"ot{wts}_{vv}")
                        if vv == 0:
                            nc.vector.tensor_scalar(ot[:, :], sl, nlzneg, None, ALU.add)
                        else:
                            nc.scalar.activation(ot[:, :], sl, AF.Identity, bias=nlzneg)
                        nc.sync.dma_start(out3[:, wts, vv * VH:(vv + 1) * VH], ot[:, :])

                def read_cc_and_resolve(rts, need_token):
                    """Read topall(rts); resolve winner token (if needed) and,
                    for rts>=1, the lagged normalizer of step rts-1 + write."""
                    topall = wk.tile([B, 4 * NCORE], F32, tag="topall", bufs=2,
                                     name=f"topall{rts}")
                    nc.sync.dma_start(
                        topall[:, :].rearrange("b (c v) -> b c v", v=4),
                        cc_top_out[rts].rearrange("(c b) v -> b c v", b=B),
                    )
                    t3 = topall[:, :].rearrange("b (c v) -> b c v", v=4)
                    newl = None
                    if need_token:
                        gv = wk.tile([B, 1], F32, tag="gv", bufs=2, name=f"gv{rts}")
                        nc.vector.tensor_reduce(gv[:, :], t3[:, :, 0],
                                                axis=mybir.AxisListType.X, op=ALU.max)
                        wtokf = wk.tile([B, NCORE], F32, tag="wtokf", bufs=2,
                                        name=f"wtokf{rts}")
                        nc.vector.scalar_tensor_tensor(wtokf[:, :], t3[:, :, 0], gv[:, :],
                                                       t3[:, :, 1], ALU.is_equal, ALU.mult)
                        wtok = wk.tile([B, 1], F32, tag="wtok", bufs=2, name=f"wtok{rts}")
                        nc.vector.tensor_reduce(wtok[:, :], wtokf[:, :],
                                                axis=mybir.AxisListType.X, op=ALU.max)
                        wtoki = wk.tile([B, 1], DT.int32, tag="wtoki", bufs=2,
                                        name=f"wtoki{rts}")
                        nc.vector.tensor_copy(wtoki[:, :], wtok[:, :])
                        lemb = wk.tile([B, E], BF, tag="lemb", bufs=2, name=f"lemb{rts}")
                        nc.gpsimd.indirect_dma_start(
                            out=lemb[:, :], out_offset=None, in_=io["embed"][:, :],
                            in_offset=bass.IndirectOffsetOnAxis(ap=wtoki[:, :1], axis=0),
                        )
                        newl = wk.tile([128, 2 * B], BF, tag="lastT", bufs=2,
                                       name=f"lastT{rts}")
                        for h in range(2):
                            pt = ps.tile([128, B], BF, tag="bank", name=f"lt{rts}_{h}")
                            nc.tensor.transpose(pt[:, :], lemb[:, h * 128:(h + 1) * 128],
                                                identB[:B, :B])
                            evict(newl[:, h * B:(h + 1) * B], pt[:, :])
                    if rts >= 1:
                        # lagged normalizer for step rts-1
                        sxt = wk.tile([B, 1], F32, tag="sxt", bufs=2, name=f"sxt{rts}")
                        nc.vector.tensor_reduce(sxt[:, :], t3[:, :, 2],
                                                axis=mybir.AxisListType.X, op=ALU.add)
                        si = wk.tile([B, 1], F32, tag="si", bufs=2, name=f"si{rts}")
                        nc.vector.tensor_copy(si[:, :], sxt[:, :].bitcast(DT.int32))
                        y0 = wk.tile([B, 1], F32, tag="y0", bufs=2, name=f"y0{rts}")
                        nc.vector.tensor_scalar(y0[:, :], si[:, :], LOGB, LOGK,
                                                ALU.subtract, ALU.mult)
                        ee = wk.tile([B, 1], F32, tag="ee", bufs=2, name=f"ee{rts}")
                        nc.scalar.activation(ee[:, :], y0[:, :], AF.Exp, scale=-1.0)
                        zz = wk.tile([B, 1], F32, tag="zz", bufs=2, name=f"zz{rts}")
                        nc.vector.tensor_mul(zz[:, :], sxt[:, :], ee[:, :])
                        nlzneg = wk.tile([B, 1], F32, tag="nlzneg", bufs=2, name=f"nlz{rts}")
                        nc.vector.scalar_tensor_tensor(nlzneg[:, :], zz[:, :], 1.0, y0[:, :],
                                                       ALU.subtract, ALU.add)
                        nc.vector.tensor_scalar_mul(nlzneg[:, :], nlzneg[:, :], -1.0)
                        write_step(rts - 1, nlzneg[:, :])
                    return newl

                xw_ans = ans_xw[:2]
                hw_ans = W["ans"][1]
                for ts in range(n_dec):
                    # --- recurrent-path matmuls first (run inside CC window) ---
                    drz = ps.tile([128, 4 * B], F32, tag="bank", name=f"drz{ts}")
                    dnh = ps.tile([128, 2 * B], F32, tag="bank", name=f"dnh{ts}")

                    def ddst(m):
                        if m < 4:
                            return drz[:, m * B:(m + 1) * B]
                        return dnh[:, (m - 4) * B:(m - 3) * B]

                    for m in range(6):
                        for k in range(2):
                            nc.tensor.matmul(ddst(m), hw_ans[k][:, m * 128:(m + 1) * 128],
                                             hid[:, k * B:(k + 1) * B],
                                             start=(k == 0), stop=False,
                                             skip_group_check=True)
                    if ts > 0:
                        lastT = read_cc_and_resolve(ts - 1, need_token=True)
                    for m in range(6):
                        for k in range(2):
                            nc.tensor.matmul(ddst(m), xw_ans[k][:, m * 128:(m + 1) * 128],
                                             lastT[:, k * B:(k + 1) * B],
                                             start=False, stop=(k == 1),
                                             skip_group_check=True)
                    hid = gru_ew(drz, dnh, "ans", hid[:, :], B,
                                 giq63[:, 0:4, :], giq63[:, 4:6, :],
                                 name=f"a{ts}_", sig_direct=False)

                    # --- fc + scan (copies on Act, chunk maxes on DVE) ---
                    store = dp.tile([B, VS], F32, tag="lst", bufs=3, name=f"lst{ts}")
                    stores[ts] = store[:, :]
                    Mt = wk.tile([B, NCHUNK], F32, tag="Mt", bufs=2, name=f"Mt{ts}")
                    pls = []
                    for c in range(NCHUNK):
                        pl = ps.tile([B, CHW], F32, tag="bank", name=f"pl{ts}_{c}")
                        pls.append(pl)
                        for k in range(2):
                            nc.tensor.matmul(pl[:, :], hid[:, k * B:(k + 1) * B],
                                             fcwT[k][:, c * CHW:(c + 1) * CHW],
                                             start=(k == 0), stop=(k == 1))
                        if fcb_nonzero:
                            fcbt = wk.tile([B, CHW], F32, tag="fcbt", bufs=2,
                                           name=f"fcbt{ts}_{c}")
                            nc.sync.dma_start(fcbt[:, :], io["fcb"][:, c * CHW:(c + 1) * CHW])
                            nc.vector.tensor_add(pl[:, :], pl[:, :], fcbt[:, :])
                        nc.scalar.activation(store[:, c * CHW:(c + 1) * CHW],
                                             pl[:, :], AF.Copy)
                        nc.vector.tensor_reduce(Mt[:, c:c + 1],
                                                store[:, c * CHW:(c + 1) * CHW],
                                                axis=mybir.AxisListType.X, op=ALU.max)
                    # half-maxes + argmax index per half (sum over single match)
                    Mh = wk.tile([B, 2], F32, tag="Mh", bufs=2, name=f"Mh{ts}")
                    It = wk.tile([B, 2], F32, tag="It", bufs=2, name=f"It{ts}")
                    for v in range(2):
                        nc.vector.tensor_reduce(Mh[:, v:v + 1], Mt[:, v * 4:(v + 1) * 4],
                                                axis=mybir.AxisListType.X, op=ALU.max)
                        mskh = dp.tile([B, VH], F32, tag="mskh", bufs=2,
                                       name=f"mskh{ts}_{v}")
                        nc.vector.scalar_tensor_tensor(
                            mskh[:, :], store[:, v * VH:(v + 1) * VH], Mh[:, v:v + 1],
                            iotaG[:, v * VH:(v + 1) * VH], ALU.is_equal, ALU.mult,
                            accum_out=It[:, v:v + 1])
                    gmax = wk.tile([B, 1], F32, tag="gmax", bufs=2, name=f"gmax{ts}")
                    nc.vector.tensor_reduce(gmax[:, :], Mh[:, :], axis=mybir.AxisListType.X,
                                            op=ALU.max)
                    wsel = wk.tile([B, 2], F32, tag="wsel", bufs=2, name=f"wsel{ts}")
                    nc.vector.scalar_tensor_tensor(wsel[:, :], Mh[:, :], gmax[:, :],
                                                   It[:, :], ALU.is_equal, ALU.mult)
                    tokf = wk.tile([B, 1], F32, tag="tokf", bufs=2, name=f"tokf{ts}")
                    nc.vector.tensor_reduce(tokf[:, :], wsel[:, :], axis=mybir.AxisListType.X,
                                            op=ALU.max)
                    pack = wk.tile([B, 4], F32, tag="pack", bufs=2, name=f"pack{ts}")
                    nc.vector.tensor_copy(pack[:, 0:1], gmax[:, :])
                    nc.vector.tensor_add(pack[:, 1:2], tokf[:, :], vofft[:, :])
                    if sxs_prev is not None:
                        nc.vector.tensor_copy(pack[:, 2:3], sxs_prev)
                    else:
                        nc.vector.memset(pack[:, 2:3], 0.0)
                    nc.vector.memset(pack[:, 3:4], 0.0)
                    nc.sync.dma_start(cc_top_in[ts][:, :], pack[:, :])
                    nc.gpsimd.collective_compute("AllGather", ALU.bypass,
                                                 ins=[cc_top_in[ts][:, :]],
                                                 outs=[cc_top_out[ts][:, :]], replica_groups=rg)

                    # --- lagged exp/sum: runs inside this step's CC window ---
                    sx = wk.tile([B, NCHUNK], F32, tag="sx", bufs=2, name=f"sx{ts}")
                    for c in range(NCHUNK):
                        dump = dp.tile([B, CHW], BF, tag="dump", bufs=4, name=f"dump{ts}_{c}")
                        nc.scalar.activation(dump[:, :], pls[c][:, :], AF.Exp,
                                             accum_out=sx[:, c:c + 1])
                    sxs = wk.tile([B, 1], F32, tag="sxs", bufs=2, name=f"sxs{ts}")
                    nc.vector.tensor_reduce(sxs[:, :], sx[:, :], axis=mybir.AxisListType.X,
                                            op=ALU.add)
                    sxs_prev = sxs[:, :]

                # final collective: ship the last step's expsum
                packF = wk.tile([B, 4], F32, tag="pack", bufs=2, name="packF")
                nc.vector.memset(packF[:, 0:2], 0.0)
                nc.vector.tensor_copy(packF[:, 2:3], sxs_prev)
                nc.vector.memset(packF[:, 3:4], 0.0)
                nc.sync.dma_start(cc_top_in[n_dec][:, :], packF[:, :])
                nc.gpsimd.collective_compute("AllGather", ALU.bypass,
                                             ins=[cc_top_in[n_dec][:, :]],
                                             outs=[cc_top_out[n_dec][:, :]], replica_groups=rg)
                read_cc_and_resolve(n_dec - 1, need_token=False)
                read_cc_and_resolve(n_dec, need_token=False)

    nc.finalize()
    return nc


def prep_inputs(inputs):
    """Host-side shard/pack. Returns in_maps list for the 8 cores."""
    f32 = np.float32
    bfnp = DT.np(BF)
    emb = np.asarray(inputs["embed_w"], dtype=f32).astype(bfnp)
    packs = {}
    for g in GK:
        wih = np.asarray(inputs[f"{g}_wih"], dtype=f32)
        whh = np.asarray(inputs[f"{g}_whh"], dtype=f32)
        bih = np.asarray(inputs[f"{g}_bih"], dtype=f32)
        bhh = np.asarray(inputs[f"{g}_bhh"], dtype=f32)
        packs[f"{g}_wihT"] = np.ascontiguousarray(wih.T).astype(bfnp)
        packs[f"{g}_whhT"] = np.ascontiguousarray(whh.T).astype(bfnp)
        brz6 = np.empty((128, 6), f32)
        for m in range(4):
            brz6[:, m] = bih[m * 128:(m + 1) * 128] + bhh[m * 128:(m + 1) * 128]
        for hh in range(2):
            brz6[:, 4 + hh] = bih[512 + hh * 128:512 + (hh + 1) * 128]
        packs[f"{g}_brz6"] = brz6
        packs[f"{g}_bhn"] = np.ascontiguousarray(bhh[512:768].reshape(2, 128).T)
    packs["g1T"] = np.ascontiguousarray(np.asarray(inputs["gate_w1"], f32).T).astype(bfnp)
    packs["g2T"] = np.ascontiguousarray(np.asarray(inputs["gate_w2"], f32).T).astype(bfnp)
    packs["gb1"] = np.ascontiguousarray(np.asarray(inputs["gate_b1"], f32).reshape(2, 128).T)
    packs["gb2"] = np.asarray(inputs["gate_b2"], f32).reshape(1, 1)
    # memory-GRU constant bias tiles (x side has no hoisted projection)
    memb = packs["mem_brz6"]
    packs["memrz"] = np.ascontiguousarray(np.repeat(memb[:, 0:4], BB, axis=1), f32)
    packs["membin"] = np.ascontiguousarray(np.repeat(memb[:, 4:6], BB, axis=1), f32)
    fcwT = np.ascontiguousarray(np.asarray(inputs["fc_w"], f32).T)
    fcb = np.asarray(inputs["fc_b"], f32)
    fcb_nonzero = bool(np.any(fcb != 0))
    e1 = np.asarray(inputs["embed_w"], f32)[SEQBEGIN].astype(bfnp)
    l0 = np.empty((128, 2 * B), bfnp)
    for k in range(2):
        l0[:, k * B:(k + 1) * B] = np.tile(e1[k * 128:(k + 1) * 128][:, None], (1, B))
    allfacts = np.asarray(inputs["allfacts"], np.int32)
    questions = np.asarray(inputs["questions"], np.int32)

    in_maps = []
    for j in range(NCORE):
        m = dict(packs)
        m["embed"] = emb
        m["fcwT"] = np.ascontiguousarray(fcwT[:, j * VS:(j + 1) * VS]).astype(bfnp)
        if fcb_nonzero:
            m["fcb"] = np.ascontiguousarray(np.tile(fcb[None, j * VS:(j + 1) * VS], (B, 1)))
        m["l0"] = l0
        m["voff"] = np.full((B, 1), j * VS, f32)
        m["facts_idx"] = np.ascontiguousarray(
            allfacts[j * BB:(j + 1) * BB].reshape(NSEQ, FL).T.reshape(-1, 1))
        m["q_idx"] = np.ascontiguousarray(
            questions[j * BB:(j + 1) * BB].reshape(BB, QL).T.reshape(-1, 1))
        in_maps.append(m)
    return in_maps, fcb_nonzero


_CACHE = {}


def kernel(**inputs):
    alen = int(inputs["alen"])
    in_maps, fcb_nonzero = prep_inputs(inputs)
    key = (alen, fcb_nonzero)
    if key not in _CACHE:
        _CACHE[key] = build_nc(alen, fcb_nonzero)
    nc = _CACHE[key]
    res = run_bass_kernel_spmd(nc, in_maps, core_ids=list(range(NCORE)))
    out = np.concatenate([res.results[j]["out_logp"] for j in range(NCORE)], axis=1)
    return out.astype(np.float32)
